# revision 19
# baseline (speedup 1.0000x reference)
"""MiniTransformer layer on 8 TRN2 NeuronCores.

Strategy: data-parallel over batch (B=8 -> one batch element per core, no
collectives). Per core, one full transformer block over [S=2048, D=1024].

All matmuls run in bf16 (fp32 PSUM accumulation). Host-side layout prep:
weights pre-transposed/cast to bf16; the q/k projections are folded into a
single matrix G = Wk^T @ Wq so scoresT = (x@G) @ x^T (saves one projection
and keeps both score operands in transposed layout -> softmax needs no
on-chip transposes; max-subtraction is skipped since |scores/sqrt(D)| < 3).

Per-core flow:
  phase1:  zT[d',s] = G^T-contraction over xT;  v[s,e] = x @ Wv^T
  chunks of 256 tokens:
    scoresT[sk, sq] = zT^T-tiles @ xT-chunk     (PSUM f32)
    PT = exp(scoresT/32) bf16                   (ScalarE, no max needed)
    attn = PT^T @ v ; den = PT^T @ ones         (PE, accumulated over sk)
    h = attn/den + x ; LN1                      (DVE)
    hT via PE transpose; LN1 affine folded into the PSUM->SBUF copy (DVE)
    uT = W1T^T @ hT ; relu(+b1)                 (PE + ScalarE)
    ff = uT^T @ W2T ; + (g1*t1+be1+b2) ; LN2    (PE + DVE/GpSimd)
"""

import sys

try:
    import concourse.bass as bass
except ImportError:  # pragma: no cover - fallback when sitecustomize absent
    sys.path.insert(0, "/opt/trn_rl_repo")
    import concourse.bass as bass

import numpy as np
import ml_dtypes

import concourse.mybir as mybir
import concourse.tile as tile
from concourse import bacc
from concourse.bass import ts
from concourse.bass_utils import run_bass_kernel_spmd
from concourse.masks import make_identity

AF = mybir.ActivationFunctionType
ALU = mybir.AluOpType
F32 = mybir.dt.float32
BF16 = mybir.dt.bfloat16
BF16_NP = ml_dtypes.bfloat16

P = 128
D = 1024
H = 2048
E = D
ND = D // P            # 8 d-tiles
NH = H // P            # 16 h-tiles
CW = 256               # s-chunk width
M2 = CW // P           # 2 m-subtiles per chunk
EPS = 1e-5
INV_SQRT_D = 1.0 / 32.0


def build_nc(S=2048):
    NS = S // P            # s-tiles
    NCH = S // CW          # chunks

    nc = bacc.Bacc("TRN2", target_bir_lowering=False, debug=False, num_devices=8)

    x_res = nc.dram_tensor("x_res", [S, D], F32, kind="ExternalInput").ap()
    xT = nc.dram_tensor("xT", [D, S], BF16, kind="ExternalInput").ap()
    G = nc.dram_tensor("G", [D, D], BF16, kind="ExternalInput").ap()
    WvT = nc.dram_tensor("WvT", [D, E], BF16, kind="ExternalInput").ap()
    W1T = nc.dram_tensor("W1T", [D, H], BF16, kind="ExternalInput").ap()
    W2T = nc.dram_tensor("W2T", [H, D], BF16, kind="ExternalInput").ap()
    b1 = nc.dram_tensor("b1", [H], F32, kind="ExternalInput").ap()
    g1 = nc.dram_tensor("g1", [D], F32, kind="ExternalInput").ap()
    be1 = nc.dram_tensor("be1", [D], F32, kind="ExternalInput").ap()
    c1 = nc.dram_tensor("c1", [D], F32, kind="ExternalInput").ap()  # be1 + b2
    g2 = nc.dram_tensor("g2", [D], F32, kind="ExternalInput").ap()
    be2 = nc.dram_tensor("be2", [D], F32, kind="ExternalInput").ap()
    out = nc.dram_tensor("out", [S, D], F32, kind="ExternalOutput").ap()

    def bcast(ap_1d, n):
        return bass.AP(tensor=ap_1d.tensor, offset=ap_1d.offset, ap=[[0, P], [1, n]])

    def col(ap_1d, j):
        return ap_1d.rearrange("(a b) -> a b", b=1)[ts(j, P), :]

    with tile.TileContext(nc) as tc:
        with (
            tc.tile_pool(name="p256", bufs=3, space="PSUM") as p256,
            tc.tile_pool(name="p512", bufs=3, space="PSUM") as p512,
            tc.tile_pool(name="ptr", bufs=1, space="PSUM") as ptr,
            tc.tile_pool(name="persist", bufs=1) as persist,
            tc.tile_pool(name="scal", bufs=24) as scal,
            tc.tile_pool(name="stats", bufs=8) as stats,
            tc.tile_pool(name="dsbp", bufs=2) as dsbp,
        ):
            # ---- constants ----
            ident = persist.tile([P, P], F32, tag="ident", name="ident")
            make_identity(nc, ident)
            ones_bf = persist.tile([P, 1], BF16, tag="ones", name="ones_bf")
            nc.vector.memset(ones_bf, 1.0)
            eps_t = persist.tile([P, 1], F32, tag="eps", name="eps_t")
            nc.vector.memset(eps_t, EPS)
            g1bc = persist.tile([P, D], BF16, tag="g1bc", name="g1bc")
            nc.gpsimd.dma_start(out=g1bc, in_=bcast(g1, D))
            c1bc = persist.tile([P, D], F32, tag="c1bc", name="c1bc")
            nc.gpsimd.dma_start(out=c1bc, in_=bcast(c1, D))
            g2bc = persist.tile([P, D], BF16, tag="g2bc", name="g2bc")
            nc.gpsimd.dma_start(out=g2bc, in_=bcast(g2, D))
            be2bc = persist.tile([P, D], BF16, tag="be2bc", name="be2bc")
            nc.gpsimd.dma_start(out=be2bc, in_=bcast(be2, D))
            b1col = []
            for n in range(NH):
                t = persist.tile([P, 1], F32, tag=f"b1c{n}", name=f"b1col{n}")
                nc.gpsimd.dma_start(out=t, in_=col(b1, n))
                b1col.append(t)
            g1col, be1col = [], []
            for j in range(ND):
                t = persist.tile([P, 1], F32, tag=f"g1c{j}", name=f"g1col{j}")
                nc.gpsimd.dma_start(out=t, in_=col(g1, j))
                g1col.append(t)
                t = persist.tile([P, 1], F32, tag=f"be1c{j}", name=f"be1col{j}")
                nc.gpsimd.dma_start(out=t, in_=col(be1, j))
                be1col.append(t)

            # ---- persistent tensors ----
            zT = [persist.tile([P, S], BF16, tag=f"zT{i}", name=f"zT{i}")
                  for i in range(ND)]
            v = [persist.tile([P, E], BF16, tag=f"v{t}", name=f"v{t}")
                 for t in range(NS)]
            # W1T packed resident: [p, j, h] with d = j*128+p
            w1p = persist.tile([P, ND, H], BF16, tag="w1p", name="w1p")

            # ===== phase 1: zT and v, streaming xT chunk-by-chunk so the
            # first matmul starts after ~2.5MB of DMA instead of ~12MB =====
            with (
                tc.tile_pool(name="ph1", bufs=1) as ph1,
                tc.tile_pool(name="xp1", bufs=20) as xp1,
            ):
                gp = ph1.tile([P, ND, D], BF16, tag="gp", name="gp")
                for j in range(ND):
                    nc.sync.dma_start(out=gp[:, j, :], in_=G[ts(j, P), :])
                # first zT group only needs gp + xs(0); wv queued after xs(0)
                # so the first matmuls start ~2MB of DMA earlier
                xs0 = []
                for j in range(ND):
                    t = xp1.tile([P, CW], BF16, tag="xs", name=f"xs0_{j}")
                    nc.sync.dma_start(out=t, in_=xT[ts(j, P), ts(0, CW)])
                    xs0.append(t)
                # wv in two half-loads so v(sc=0, ec=0) starts ~1MB of DMA sooner
                wv_sb = [ph1.tile([P, E], BF16, tag=f"wv{j}", name=f"wv{j}")
                         for j in range(ND)]
                for ec in range(2):
                    for j in range(ND):
                        nc.sync.dma_start(out=wv_sb[j][:, ts(ec, 512)],
                                          in_=WvT[ts(j, P), ts(ec, 512)])

                for sc in range(S // CW):
                    if sc == 0:
                        xs = xs0
                    else:
                        xs = []
                        for j in range(ND):
                            t = xp1.tile([P, CW], BF16, tag="xs", name=f"xs{sc}_{j}")
                            nc.sync.dma_start(out=t, in_=xT[ts(j, P), ts(sc, CW)])
                            xs.append(t)
                    # zT[i][:, sc] = sum_j G[j-block, i-slice]^T @ xT[j][:, sc]
                    for i in range(ND):
                        ps = p256.tile([P, CW], F32, tag="mm", name=f"zps{i}_{sc}")
                        for j in range(ND):
                            nc.tensor.matmul(ps, gp[:, j, ts(i, P)], xs[j],
                                             start=(j == 0), stop=(j == ND - 1))
                        nc.vector.tensor_copy(out=zT[i][:, ts(sc, CW)], in_=ps)
                    # v[t][:, ec] = sum_j xT[j][:, t-slice]^T @ WvT[j][:, ec]
                    for tl in range(M2):
                        t_ = sc * M2 + tl
                        for ec in range(2):
                            ps = p512.tile([P, 512], F32, tag="mm",
                                           name=f"vps{t_}_{ec}")
                            for j in range(ND):
                                nc.tensor.matmul(ps, xs[j][:, ts(tl, P)],
                                                 wv_sb[j][:, ts(ec, 512)],
                                                 start=(j == 0), stop=(j == ND - 1))
                            nc.vector.tensor_copy(out=v[t_][:, ts(ec, 512)], in_=ps)
                # W1T needed from body 1 onwards; emit after the phase-1 DMAs
                for j in range(ND):
                    nc.sync.dma_start(out=w1p[:, j, :], in_=W1T[ts(j, P), :])

            # ======== chunk loop, software-pipelined: FFN runs one chunk
            # behind attention so the LN1->transpose dependency chain hides
            # under FFN(c-1)'s PE work (no PE stall, no HAM re-throttle) ====
            with (
                tc.tile_pool(name="xTc", bufs=9) as xTcp,
                tc.tile_pool(name="PT", bufs=17) as PTp,
                tc.tile_pool(name="hTc", bufs=18) as hTcp,
                tc.tile_pool(name="uT", bufs=17) as uTp,
                tc.tile_pool(name="w2s", bufs=10) as w2sp,
                tc.tile_pool(name="xmp", bufs=3) as xmp,
                tc.tile_pool(name="f32s", bufs=10) as f32sp,
            ):
                hc_prev = hr_prev = None
                for c in range(NCH + 1):
                    hp_cur, hr_cur = [], []
                    if c < NCH:
                        # ---- prefetches ----
                        xc = []
                        for j in range(ND):
                            t = xTcp.tile([P, CW], BF16, tag="xc", name=f"xc{c}_{j}")
                            nc.sync.dma_start(out=t, in_=xT[ts(j, P), ts(c, CW)])
                            xc.append(t)
                        xm = []
                        for m in range(M2):
                            t = xmp.tile([P, D], F32, tag="xm", name=f"xm{c}_{m}")
                            nc.sync.dma_start(out=t, in_=x_res[ts(c * M2 + m, P), :])
                            xm.append(t)
                        # ---- scoresT + exp ----
                        pt = []
                        for t_ in range(NS):
                            ps = p256.tile([P, CW], F32, tag="mm", name=f"sps{c}_{t_}")
                            for i in range(ND):
                                nc.tensor.matmul(ps, zT[i][:, ts(t_, P)], xc[i],
                                                 start=(i == 0), stop=(i == ND - 1))
                            t = PTp.tile([P, CW], BF16, tag="pt", name=f"pt{c}_{t_}")
                            nc.scalar.activation(out=t, in_=ps, func=AF.Exp,
                                                 scale=INV_SQRT_D)
                            pt.append(t)
                        # ---- batched denominators: ones stationary (1-col
                        # LDW ~ free), PT moving -> den for both m at once in
                        # [1, CW] layout; tiny transpose back to [P, 1] ----
                        pd1 = ptr.tile([1, CW], F32, tag="den", name=f"pd{c}")
                        for t_ in range(NS):
                            nc.tensor.matmul(pd1, ones_bf, pt[t_],
                                             start=(t_ == 0), stop=(t_ == NS - 1))
                        den_sb = dsbp.tile([1, CW], F32, tag="dsb", name=f"dsb{c}")
                        nc.vector.tensor_copy(out=den_sb, in_=pd1)
                        rs = []
                        for m in range(M2):
                            rp = ptr.tile([P, 1], F32, tag="tr", name=f"rtp{c}_{m}")
                            nc.tensor.matmul(rp, den_sb[0:1, ts(m, P)],
                                             ident[0:1, 0:1], is_transpose=True)
                            r = scal.tile([P, 1], F32, tag="r", name=f"r{c}_{m}")
                            nc.vector.reciprocal(r, rp)
                            rs.append(r)
                        # ---- PV; normalize + residual + LN1 ----
                        for m in range(M2):
                            pa = [p512.tile([P, 512], F32, tag="mm",
                                            name=f"pa{c}_{m}_{ec}")
                                  for ec in range(2)]
                            for t_ in range(NS):
                                lhs = pt[t_][:, ts(m, P)]
                                nc.tensor.matmul(pa[0], lhs, v[t_][:, 0:512],
                                                 start=(t_ == 0), stop=(t_ == NS - 1))
                                nc.tensor.matmul(pa[1], lhs, v[t_][:, 512:1024],
                                                 start=(t_ == 0), stop=(t_ == NS - 1))
                            r = rs[m]
                            hp = f32sp.tile([P, D], F32, tag="f32", name=f"hp{c}_{m}")
                            for ec in range(2):
                                nc.vector.scalar_tensor_tensor(
                                    out=hp[:, ts(ec, 512)], in0=pa[ec], scalar=r,
                                    in1=xm[m][:, ts(ec, 512)],
                                    op0=ALU.mult, op1=ALU.add)
                            # LN1
                            st = stats.tile([P, 2, 6], F32, tag="st", name=f"st{c}_{m}")
                            for hf in range(2):
                                nc.vector.bn_stats(out=st[:, hf, :],
                                                   in_=hp[:, ts(hf, 512)])
                            mv = scal.tile([P, 2], F32, tag="mv", name=f"mv{c}_{m}")
                            nc.vector.bn_aggr(out=mv, in_=st)
                            rstd = scal.tile([P, 1], F32, tag="rstd",
                                             name=f"rstd{c}_{m}")
                            nc.scalar.activation(out=rstd, in_=mv[:, 1:2],
                                                 func=AF.Sqrt, bias=eps_t)
                            nc.vector.reciprocal(rstd, rstd)
                            nc.vector.tensor_scalar(out=hp, in0=hp,
                                                    scalar1=mv[:, 0:1], scalar2=rstd,
                                                    op0=ALU.subtract, op1=ALU.mult)
                            hp_cur.append(hp)
                            # natural-layout affine residual: g1*t1 + (be1+b2)
                            hrm = f32sp.tile([P, D], F32, tag="f32",
                                             name=f"hr{c}_{m}")
                            nc.gpsimd.tensor_mul(hrm, hp, g1bc)
                            nc.gpsimd.tensor_add(hrm, hrm, c1bc)
                            hr_cur.append(hrm)
                    if c > 0:
                        cp = c - 1
                        # ---- FFN1(cp) ----
                        ut = []
                        for n in range(NH):
                            ps = p256.tile([P, CW], F32, tag="mm", name=f"ups{cp}_{n}")
                            for j in range(ND):
                                nc.tensor.matmul(ps, w1p[:, j, ts(n, P)], hc_prev[j],
                                                 start=(j == 0), stop=(j == ND - 1))
                            t = uTp.tile([P, CW], BF16, tag="ut", name=f"ut{cp}_{n}")
                            nc.scalar.activation(out=t, in_=ps, func=AF.Relu,
                                                 bias=b1col[n])
                            ut.append(t)
                        # ---- FFN2(cp): dc-split (2 psums live instead of 4);
                        # dc0's residual-add overlaps dc1's matmuls ----
                        u2 = [f32sp.tile([P, D], F32, tag="f32", name=f"u2{cp}_{m}")
                              for m in range(M2)]
                        for dc in range(2):
                            psm = [p512.tile([P, 512], F32, tag="mm",
                                             name=f"fps{cp}_{dc}_{m}")
                                   for m in range(M2)]
                            for n in range(NH):
                                w2h = w2sp.tile([P, 512], BF16, tag="w2",
                                                name=f"w2_{cp}_{dc}_{n}")
                                nc.sync.dma_start(out=w2h,
                                                  in_=W2T[ts(n, P), ts(dc, 512)])
                                for m in range(M2):
                                    nc.tensor.matmul(psm[m], ut[n][:, ts(m, P)],
                                                     w2h,
                                                     start=(n == 0),
                                                     stop=(n == NH - 1))
                            for m in range(M2):
                                nc.vector.scalar_tensor_tensor(
                                    out=u2[m][:, ts(dc, 512)], in0=psm[m],
                                    scalar=1.0, in1=hr_prev[m][:, ts(dc, 512)],
                                    op0=ALU.mult, op1=ALU.add)
                        for m in range(M2):
                            sq = cp * M2 + m
                            st = stats.tile([P, 2, 6], F32, tag="st",
                                            name=f"st2{cp}_{m}")
                            for hf in range(2):
                                nc.vector.bn_stats(out=st[:, hf, :],
                                                   in_=u2[m][:, ts(hf, 512)])
                            mv = scal.tile([P, 2], F32, tag="mv", name=f"mv2{cp}_{m}")
                            nc.vector.bn_aggr(out=mv, in_=st)
                            rstd = scal.tile([P, 1], F32, tag="rstd",
                                             name=f"rstd2{cp}_{m}")
                            nc.scalar.activation(out=rstd, in_=mv[:, 1:2],
                                                 func=AF.Sqrt, bias=eps_t)
                            nc.vector.reciprocal(rstd, rstd)
                            nc.vector.tensor_scalar(out=u2[m], in0=u2[m],
                                                    scalar1=mv[:, 0:1], scalar2=rstd,
                                                    op0=ALU.subtract, op1=ALU.mult)
                            ot = f32sp.tile([P, D], F32, tag="f32", name=f"ot{cp}_{m}")
                            # last body: DVE for the tail affine (gpsimd is ~3x
                            # slower and would sit on the critical path)
                            eng = nc.vector if cp == NCH - 1 else nc.gpsimd
                            eng.tensor_mul(ot, u2[m], g2bc)
                            eng.tensor_add(ot, ot, be2bc)
                            nc.sync.dma_start(out=out[ts(sq, P), :], in_=ot)
                    if c < NCH:
                        # ---- hT transposes (LN1 affine folded into DVE copy);
                        # emitted after FFN(c-1) so the LN1 chain is hidden ----
                        hc = [hTcp.tile([P, CW], BF16, tag="hc", name=f"hc{c}_{j}")
                              for j in range(ND)]
                        for m in range(M2):
                            for j in range(ND):
                                pst = ptr.tile([P, P], F32, tag="tr",
                                               name=f"tr{c}_{m}_{j}")
                                nc.tensor.transpose(pst, hp_cur[m][:, ts(j, P)], ident)
                                if c == NCH - 1:
                                    # last chunk: DVE is busy with LN2(c-1);
                                    # use idle ScalarE so the tail FFN isn't
                                    # gated on these copies
                                    nc.scalar.activation(out=hc[j][:, ts(m, P)],
                                                         in_=pst, func=AF.Identity,
                                                         bias=be1col[j],
                                                         scale=g1col[j])
                                else:
                                    nc.vector.tensor_scalar(out=hc[j][:, ts(m, P)],
                                                            in0=pst,
                                                            scalar1=g1col[j],
                                                            scalar2=be1col[j],
                                                            op0=ALU.mult,
                                                            op1=ALU.add)
                        hc_prev, hr_prev = hc, hr_cur

    nc.compile()
    return nc


_CACHE = {}


def _get_nc(S):
    if S not in _CACHE:
        _CACHE[S] = build_nc(S)
    return _CACHE[S]


def kernel(x, Wq, Wk, Wv, W1, b1, W2, b2, g1, be1, g2, be2):
    x = np.asarray(x, np.float32)
    B, S, D_ = x.shape
    nc = _get_nc(S)

    def bft(a):  # transpose + cast to bf16, contiguous
        return np.ascontiguousarray(np.asarray(a, np.float32).T).astype(BF16_NP)

    Gm = (np.asarray(Wk, np.float32).T @ np.asarray(Wq, np.float32)).astype(BF16_NP)
    shared = {
        "G": Gm, "WvT": bft(Wv), "W1T": bft(W1), "W2T": bft(W2),
        "b1": np.asarray(b1, np.float32),
        "g1": np.asarray(g1, np.float32),
        "be1": np.asarray(be1, np.float32),
        "c1": np.asarray(be1, np.float32) + np.asarray(b2, np.float32),
        "g2": np.asarray(g2, np.float32),
        "be2": np.asarray(be2, np.float32),
    }
    in_maps = []
    for b in range(B):
        m = dict(shared)
        m["x_res"] = np.ascontiguousarray(x[b])
        m["xT"] = bft(x[b])
        in_maps.append(m)

    res = run_bass_kernel_spmd(nc, in_maps, core_ids=list(range(B)))
    return np.stack([np.asarray(res.results[b]["out"], np.float32)
                     for b in range(B)], axis=0)


# revision 23
# speedup vs baseline: 1.0103x; 1.0103x over previous
"""MiniTransformer layer on 8 TRN2 NeuronCores.

Strategy: data-parallel over batch (B=8 -> one batch element per core, no
collectives). Per core, one full transformer block over [S=2048, D=1024].

All matmuls run in bf16 (fp32 PSUM accumulation). Host-side layout prep:
weights pre-transposed/cast to bf16; the q/k projections are folded into a
single matrix G = Wk^T @ Wq so scoresT = (x@G) @ x^T (saves one projection
and keeps both score operands in transposed layout -> softmax needs no
on-chip transposes; max-subtraction is skipped since |scores/sqrt(D)| < 3).

Per-core flow:
  phase1:  zT[d',s] = G^T-contraction over xT;  v[s,e] = x @ Wv^T
  chunks of 256 tokens:
    scoresT[sk, sq] = zT^T-tiles @ xT-chunk     (PSUM f32)
    PT = exp(scoresT/32) bf16                   (ScalarE, no max needed)
    attn = PT^T @ v ; den = PT^T @ ones         (PE, accumulated over sk)
    h = attn/den + x ; LN1                      (DVE)
    hT via PE transpose; LN1 affine folded into the PSUM->SBUF copy (DVE)
    uT = W1T^T @ hT ; relu(+b1)                 (PE + ScalarE)
    ff = uT^T @ W2T ; + (g1*t1+be1+b2) ; LN2    (PE + DVE/GpSimd)
"""

import sys

try:
    import concourse.bass as bass
except ImportError:  # pragma: no cover - fallback when sitecustomize absent
    sys.path.insert(0, "/opt/trn_rl_repo")
    import concourse.bass as bass

import numpy as np
import ml_dtypes

import concourse.mybir as mybir
import concourse.tile as tile
from concourse import bacc
from concourse.bass import ts
from concourse.bass_utils import run_bass_kernel_spmd
from concourse.masks import make_identity

AF = mybir.ActivationFunctionType
ALU = mybir.AluOpType
F32 = mybir.dt.float32
BF16 = mybir.dt.bfloat16
BF16_NP = ml_dtypes.bfloat16

P = 128
D = 1024
H = 2048
E = D
ND = D // P            # 8 d-tiles
NH = H // P            # 16 h-tiles
CW = 256               # s-chunk width
M2 = CW // P           # 2 m-subtiles per chunk
EPS = 1e-5
INV_SQRT_D = 1.0 / 32.0


def build_nc(S=2048):
    NS = S // P            # s-tiles
    NCH = S // CW          # chunks

    nc = bacc.Bacc("TRN2", target_bir_lowering=False, debug=False, num_devices=8)

    x_res = nc.dram_tensor("x_res", [S, D], F32, kind="ExternalInput").ap()
    xT = nc.dram_tensor("xT", [D, S], BF16, kind="ExternalInput").ap()
    G = nc.dram_tensor("G", [D, D], BF16, kind="ExternalInput").ap()
    WvT = nc.dram_tensor("WvT", [D, E], BF16, kind="ExternalInput").ap()
    W1T = nc.dram_tensor("W1T", [D, H], BF16, kind="ExternalInput").ap()
    W2T = nc.dram_tensor("W2T", [H, D], BF16, kind="ExternalInput").ap()
    b1 = nc.dram_tensor("b1", [H], F32, kind="ExternalInput").ap()
    g1 = nc.dram_tensor("g1", [D], F32, kind="ExternalInput").ap()
    be1 = nc.dram_tensor("be1", [D], F32, kind="ExternalInput").ap()
    c1 = nc.dram_tensor("c1", [D], F32, kind="ExternalInput").ap()  # be1 + b2
    g2 = nc.dram_tensor("g2", [D], F32, kind="ExternalInput").ap()
    be2 = nc.dram_tensor("be2", [D], F32, kind="ExternalInput").ap()
    out = nc.dram_tensor("out", [S, D], F32, kind="ExternalOutput").ap()

    def bcast(ap_1d, n):
        return bass.AP(tensor=ap_1d.tensor, offset=ap_1d.offset, ap=[[0, P], [1, n]])

    def col(ap_1d, j):
        return ap_1d.rearrange("(a b) -> a b", b=1)[ts(j, P), :]

    with tile.TileContext(nc) as tc:
        with (
            tc.tile_pool(name="p256", bufs=2, space="PSUM") as p256,
            tc.tile_pool(name="p512", bufs=4, space="PSUM") as p512,
            tc.tile_pool(name="ptr", bufs=1, space="PSUM") as ptr,
            tc.tile_pool(name="persist", bufs=1) as persist,
            tc.tile_pool(name="scal", bufs=24) as scal,
            tc.tile_pool(name="stats", bufs=8) as stats,
            tc.tile_pool(name="dsbp", bufs=2) as dsbp,
        ):
            # ---- constants ----
            ident = persist.tile([P, P], F32, tag="ident", name="ident")
            make_identity(nc, ident)
            ones_bf = persist.tile([P, 1], BF16, tag="ones", name="ones_bf")
            nc.vector.memset(ones_bf, 1.0)
            eps_t = persist.tile([P, 1], F32, tag="eps", name="eps_t")
            nc.vector.memset(eps_t, EPS)
            g1bc = persist.tile([P, D], BF16, tag="g1bc", name="g1bc")
            nc.gpsimd.dma_start(out=g1bc, in_=bcast(g1, D))
            c1bc = persist.tile([P, D], F32, tag="c1bc", name="c1bc")
            nc.gpsimd.dma_start(out=c1bc, in_=bcast(c1, D))
            g2bc = persist.tile([P, D], BF16, tag="g2bc", name="g2bc")
            nc.gpsimd.dma_start(out=g2bc, in_=bcast(g2, D))
            be2bc = persist.tile([P, D], BF16, tag="be2bc", name="be2bc")
            nc.gpsimd.dma_start(out=be2bc, in_=bcast(be2, D))
            b1col = []
            for n in range(NH):
                t = persist.tile([P, 1], F32, tag=f"b1c{n}", name=f"b1col{n}")
                nc.gpsimd.dma_start(out=t, in_=col(b1, n))
                b1col.append(t)
            g1col, be1col = [], []
            for j in range(ND):
                t = persist.tile([P, 1], F32, tag=f"g1c{j}", name=f"g1col{j}")
                nc.gpsimd.dma_start(out=t, in_=col(g1, j))
                g1col.append(t)
                t = persist.tile([P, 1], F32, tag=f"be1c{j}", name=f"be1col{j}")
                nc.gpsimd.dma_start(out=t, in_=col(be1, j))
                be1col.append(t)

            # ---- persistent tensors ----
            zT = [persist.tile([P, S], BF16, tag=f"zT{i}", name=f"zT{i}")
                  for i in range(ND)]
            v = [persist.tile([P, E], BF16, tag=f"v{t}", name=f"v{t}")
                 for t in range(NS)]
            # W1T packed resident: [p, j, h] with d = j*128+p
            w1p = persist.tile([P, ND, H], BF16, tag="w1p", name="w1p")

            # ===== phase 1: zT and v, streaming xT chunk-by-chunk so the
            # first matmul starts after ~2.5MB of DMA instead of ~12MB =====
            with (
                tc.tile_pool(name="ph1", bufs=1) as ph1,
                tc.tile_pool(name="xp1", bufs=20) as xp1,
            ):
                gp = ph1.tile([P, ND, D], BF16, tag="gp", name="gp")
                for j in range(ND):
                    nc.sync.dma_start(out=gp[:, j, :], in_=G[ts(j, P), :])
                # first zT group only needs gp + xs(0); wv queued after xs(0)
                # so the first matmuls start ~2MB of DMA earlier
                xs0 = []
                for j in range(ND):
                    t = xp1.tile([P, CW], BF16, tag="xs", name=f"xs0_{j}")
                    nc.sync.dma_start(out=t, in_=xT[ts(j, P), ts(0, CW)])
                    xs0.append(t)
                # wv in two half-loads so v(sc=0, ec=0) starts ~1MB of DMA sooner
                wv_sb = [ph1.tile([P, E], BF16, tag=f"wv{j}", name=f"wv{j}")
                         for j in range(ND)]
                for ec in range(2):
                    for j in range(ND):
                        nc.sync.dma_start(out=wv_sb[j][:, ts(ec, 512)],
                                          in_=WvT[ts(j, P), ts(ec, 512)])

                for sc in range(S // CW):
                    if sc == 0:
                        xs = xs0
                    else:
                        xs = []
                        for j in range(ND):
                            t = xp1.tile([P, CW], BF16, tag="xs", name=f"xs{sc}_{j}")
                            nc.sync.dma_start(out=t, in_=xT[ts(j, P), ts(sc, CW)])
                            xs.append(t)
                    # zT[i][:, sc] = sum_j G[j-block, i-slice]^T @ xT[j][:, sc]
                    for i in range(ND):
                        ps = p256.tile([P, CW], F32, tag="mm", name=f"zps{i}_{sc}")
                        for j in range(ND):
                            nc.tensor.matmul(ps, gp[:, j, ts(i, P)], xs[j],
                                             start=(j == 0), stop=(j == ND - 1))
                        nc.vector.tensor_copy(out=zT[i][:, ts(sc, CW)], in_=ps)
                    # v[t][:, ec] = sum_j xT[j][:, t-slice]^T @ WvT[j][:, ec]
                    for tl in range(M2):
                        t_ = sc * M2 + tl
                        for ec in range(2):
                            ps = p512.tile([P, 512], F32, tag="mm",
                                           name=f"vps{t_}_{ec}")
                            for j in range(ND):
                                nc.tensor.matmul(ps, xs[j][:, ts(tl, P)],
                                                 wv_sb[j][:, ts(ec, 512)],
                                                 start=(j == 0), stop=(j == ND - 1))
                            nc.vector.tensor_copy(out=v[t_][:, ts(ec, 512)], in_=ps)
                # W1T needed from body 1 onwards; emit after the phase-1 DMAs
                for j in range(ND):
                    nc.sync.dma_start(out=w1p[:, j, :], in_=W1T[ts(j, P), :])

            # ======== chunk loop, software-pipelined: FFN runs one chunk
            # behind attention so the LN1->transpose dependency chain hides
            # under FFN(c-1)'s PE work (no PE stall, no HAM re-throttle) ====
            with (
                tc.tile_pool(name="xTc", bufs=9) as xTcp,
                tc.tile_pool(name="PT", bufs=17) as PTp,
                tc.tile_pool(name="hTc", bufs=18) as hTcp,
                tc.tile_pool(name="uT", bufs=17) as uTp,
                tc.tile_pool(name="w2s", bufs=6) as w2sp,
                tc.tile_pool(name="xmp", bufs=3) as xmp,
                tc.tile_pool(name="f32s", bufs=10) as f32sp,
            ):
                hc_prev = hr_prev = None
                for c in range(NCH + 1):
                    hp_cur, hr_cur = [], []
                    if c < NCH:
                        # ---- prefetches ----
                        xc = []
                        for j in range(ND):
                            t = xTcp.tile([P, CW], BF16, tag="xc", name=f"xc{c}_{j}")
                            nc.sync.dma_start(out=t, in_=xT[ts(j, P), ts(c, CW)])
                            xc.append(t)
                        xm = []
                        for m in range(M2):
                            t = xmp.tile([P, D], F32, tag="xm", name=f"xm{c}_{m}")
                            nc.sync.dma_start(out=t, in_=x_res[ts(c * M2 + m, P), :])
                            xm.append(t)
                        # ---- scoresT + exp ----
                        pt = []
                        for t_ in range(NS):
                            ps = p256.tile([P, CW], F32, tag="mm", name=f"sps{c}_{t_}")
                            for i in range(ND):
                                nc.tensor.matmul(ps, zT[i][:, ts(t_, P)], xc[i],
                                                 start=(i == 0), stop=(i == ND - 1))
                            t = PTp.tile([P, CW], BF16, tag="pt", name=f"pt{c}_{t_}")
                            nc.scalar.activation(out=t, in_=ps, func=AF.Exp,
                                                 scale=INV_SQRT_D)
                            pt.append(t)
                        # ---- batched denominators: ones stationary (1-col
                        # LDW ~ free), PT moving -> den for both m at once in
                        # [1, CW] layout; tiny transpose back to [P, 1] ----
                        pd1 = ptr.tile([1, CW], F32, tag="den", name=f"pd{c}")
                        for t_ in range(NS):
                            nc.tensor.matmul(pd1, ones_bf, pt[t_],
                                             start=(t_ == 0), stop=(t_ == NS - 1))
                        den_sb = dsbp.tile([1, CW], F32, tag="dsb", name=f"dsb{c}")
                        nc.vector.tensor_copy(out=den_sb, in_=pd1)
                        rs = []
                        for m in range(M2):
                            rp = ptr.tile([P, 1], F32, tag="tr", name=f"rtp{c}_{m}")
                            nc.tensor.matmul(rp, den_sb[0:1, ts(m, P)],
                                             ident[0:1, 0:1], is_transpose=True)
                            r = scal.tile([P, 1], F32, tag="r", name=f"r{c}_{m}")
                            nc.vector.reciprocal(r, rp)
                            rs.append(r)
                        # ---- PV; normalize + residual + LN1 ----
                        for m in range(M2):
                            pa = [p512.tile([P, 512], F32, tag="mm",
                                            name=f"pa{c}_{m}_{ec}")
                                  for ec in range(2)]
                            for t_ in range(NS):
                                lhs = pt[t_][:, ts(m, P)]
                                nc.tensor.matmul(pa[0], lhs, v[t_][:, 0:512],
                                                 start=(t_ == 0), stop=(t_ == NS - 1))
                                nc.tensor.matmul(pa[1], lhs, v[t_][:, 512:1024],
                                                 start=(t_ == 0), stop=(t_ == NS - 1))
                            r = rs[m]
                            hp = f32sp.tile([P, D], F32, tag="f32", name=f"hp{c}_{m}")
                            for ec in range(2):
                                nc.vector.scalar_tensor_tensor(
                                    out=hp[:, ts(ec, 512)], in0=pa[ec], scalar=r,
                                    in1=xm[m][:, ts(ec, 512)],
                                    op0=ALU.mult, op1=ALU.add)
                            # LN1
                            st = stats.tile([P, 2, 6], F32, tag="st", name=f"st{c}_{m}")
                            for hf in range(2):
                                nc.vector.bn_stats(out=st[:, hf, :],
                                                   in_=hp[:, ts(hf, 512)])
                            mv = scal.tile([P, 2], F32, tag="mv", name=f"mv{c}_{m}")
                            nc.vector.bn_aggr(out=mv, in_=st)
                            rstd = scal.tile([P, 1], F32, tag="rstd",
                                             name=f"rstd{c}_{m}")
                            nc.scalar.activation(out=rstd, in_=mv[:, 1:2],
                                                 func=AF.Sqrt, bias=eps_t)
                            nc.vector.reciprocal(rstd, rstd)
                            nc.vector.tensor_scalar(out=hp, in0=hp,
                                                    scalar1=mv[:, 0:1], scalar2=rstd,
                                                    op0=ALU.subtract, op1=ALU.mult)
                            hp_cur.append(hp)
                            # natural-layout affine residual: g1*t1 + (be1+b2)
                            hrm = f32sp.tile([P, D], F32, tag="f32",
                                             name=f"hr{c}_{m}")
                            nc.gpsimd.tensor_mul(hrm, hp, g1bc)
                            nc.gpsimd.tensor_add(hrm, hrm, c1bc)
                            hr_cur.append(hrm)
                    if c > 0:
                        cp = c - 1
                        # w2n prefetch first: FFN1 provides ~14us of lead time
                        w2n = []
                        for n in range(NH):
                            t = w2sp.tile([P, D], BF16, tag="w2", name=f"w2_{cp}_{n}")
                            nc.sync.dma_start(out=t, in_=W2T[ts(n, P), :])
                            w2n.append(t)
                        # ---- FFN1(cp) ----
                        ut = []
                        for n in range(NH):
                            ps = p256.tile([P, CW], F32, tag="mm", name=f"ups{cp}_{n}")
                            for j in range(ND):
                                nc.tensor.matmul(ps, w1p[:, j, ts(n, P)], hc_prev[j],
                                                 start=(j == 0), stop=(j == ND - 1))
                            t = uTp.tile([P, CW], BF16, tag="ut", name=f"ut{cp}_{n}")
                            nc.scalar.activation(out=t, in_=ps, func=AF.Relu,
                                                 bias=b1col[n])
                            ut.append(t)
                        # ---- FFN2(cp): 4 concurrent psums (dc x m) ----
                        psm = [[p512.tile([P, 512], F32, tag="mm",
                                          name=f"fps{cp}_{dc}_{m}")
                                for m in range(M2)] for dc in range(2)]
                        for n in range(NH):
                            for dc in range(2):
                                for m in range(M2):
                                    nc.tensor.matmul(psm[dc][m], ut[n][:, ts(m, P)],
                                                     w2n[n][:, ts(dc, 512)],
                                                     start=(n == 0),
                                                     stop=(n == NH - 1))
                        u2 = [f32sp.tile([P, D], F32, tag="f32", name=f"u2{cp}_{m}")
                              for m in range(M2)]
                        for dc in range(2):
                            for m in range(M2):
                                nc.vector.scalar_tensor_tensor(
                                    out=u2[m][:, ts(dc, 512)], in0=psm[dc][m],
                                    scalar=1.0, in1=hr_prev[m][:, ts(dc, 512)],
                                    op0=ALU.mult, op1=ALU.add)
                        for m in range(M2):
                            sq = cp * M2 + m
                            st = stats.tile([P, 2, 6], F32, tag="st",
                                            name=f"st2{cp}_{m}")
                            for hf in range(2):
                                nc.vector.bn_stats(out=st[:, hf, :],
                                                   in_=u2[m][:, ts(hf, 512)])
                            mv = scal.tile([P, 2], F32, tag="mv", name=f"mv2{cp}_{m}")
                            nc.vector.bn_aggr(out=mv, in_=st)
                            rstd = scal.tile([P, 1], F32, tag="rstd",
                                             name=f"rstd2{cp}_{m}")
                            nc.scalar.activation(out=rstd, in_=mv[:, 1:2],
                                                 func=AF.Sqrt, bias=eps_t)
                            nc.vector.reciprocal(rstd, rstd)
                            nc.vector.tensor_scalar(out=u2[m], in0=u2[m],
                                                    scalar1=mv[:, 0:1], scalar2=rstd,
                                                    op0=ALU.subtract, op1=ALU.mult)
                            ot = f32sp.tile([P, D], F32, tag="f32", name=f"ot{cp}_{m}")
                            # last body: DVE for the tail affine (gpsimd is ~3x
                            # slower and would sit on the critical path)
                            eng = nc.vector if cp == NCH - 1 else nc.gpsimd
                            eng.tensor_mul(ot, u2[m], g2bc)
                            eng.tensor_add(ot, ot, be2bc)
                            nc.sync.dma_start(out=out[ts(sq, P), :], in_=ot)
                    if c < NCH:
                        # ---- hT transposes (LN1 affine folded into DVE copy);
                        # emitted after FFN(c-1) so the LN1 chain is hidden ----
                        hc = [hTcp.tile([P, CW], BF16, tag="hc", name=f"hc{c}_{j}")
                              for j in range(ND)]
                        for m in range(M2):
                            for j in range(ND):
                                pst = ptr.tile([P, P], F32, tag="tr",
                                               name=f"tr{c}_{m}_{j}")
                                nc.tensor.transpose(pst, hp_cur[m][:, ts(j, P)], ident)
                                if c == NCH - 1:
                                    # last chunk: DVE is busy with LN2(c-1);
                                    # use idle ScalarE so the tail FFN isn't
                                    # gated on these copies
                                    nc.scalar.activation(out=hc[j][:, ts(m, P)],
                                                         in_=pst, func=AF.Identity,
                                                         bias=be1col[j],
                                                         scale=g1col[j])
                                else:
                                    nc.vector.tensor_scalar(out=hc[j][:, ts(m, P)],
                                                            in0=pst,
                                                            scalar1=g1col[j],
                                                            scalar2=be1col[j],
                                                            op0=ALU.mult,
                                                            op1=ALU.add)
                        hc_prev, hr_prev = hc, hr_cur

    nc.compile()
    return nc


_CACHE = {}


def _get_nc(S):
    if S not in _CACHE:
        _CACHE[S] = build_nc(S)
    return _CACHE[S]


def kernel(x, Wq, Wk, Wv, W1, b1, W2, b2, g1, be1, g2, be2):
    x = np.asarray(x, np.float32)
    B, S, D_ = x.shape
    nc = _get_nc(S)

    def bft(a):  # transpose + cast to bf16, contiguous
        return np.ascontiguousarray(np.asarray(a, np.float32).T).astype(BF16_NP)

    Gm = (np.asarray(Wk, np.float32).T @ np.asarray(Wq, np.float32)).astype(BF16_NP)
    shared = {
        "G": Gm, "WvT": bft(Wv), "W1T": bft(W1), "W2T": bft(W2),
        "b1": np.asarray(b1, np.float32),
        "g1": np.asarray(g1, np.float32),
        "be1": np.asarray(be1, np.float32),
        "c1": np.asarray(be1, np.float32) + np.asarray(b2, np.float32),
        "g2": np.asarray(g2, np.float32),
        "be2": np.asarray(be2, np.float32),
    }
    in_maps = []
    for b in range(B):
        m = dict(shared)
        m["x_res"] = np.ascontiguousarray(x[b])
        m["xT"] = bft(x[b])
        in_maps.append(m)

    res = run_bass_kernel_spmd(nc, in_maps, core_ids=list(range(B)))
    return np.stack([np.asarray(res.results[b]["out"], np.float32)
                     for b in range(B)], axis=0)


# revision 24
# speedup vs baseline: 1.0679x; 1.0570x over previous
"""MiniTransformer layer on 8 TRN2 NeuronCores.

Strategy: data-parallel over batch (B=8 -> one batch element per core, no
collectives). Per core, one full transformer block over [S=2048, D=1024].

All matmuls run in bf16 (fp32 PSUM accumulation). Host-side layout prep:
weights pre-transposed/cast to bf16; the q/k projections are folded into a
single matrix G = Wk^T @ Wq so scoresT = (x@G) @ x^T (saves one projection
and keeps both score operands in transposed layout -> softmax needs no
on-chip transposes; max-subtraction is skipped since |scores/sqrt(D)| < 3).

Per-core flow:
  phase1:  zT[d',s] = G^T-contraction over xT;  v[s,e] = x @ Wv^T
  chunks of 256 tokens:
    scoresT[sk, sq] = zT^T-tiles @ xT-chunk     (PSUM f32)
    PT = exp(scoresT/32) bf16                   (ScalarE, no max needed)
    attn = PT^T @ v ; den = PT^T @ ones         (PE, accumulated over sk)
    h = attn/den + x ; LN1                      (DVE)
    hT via PE transpose; LN1 affine folded into the PSUM->SBUF copy (DVE)
    uT = W1T^T @ hT ; relu(+b1)                 (PE + ScalarE)
    ff = uT^T @ W2T ; + (g1*t1+be1+b2) ; LN2    (PE + DVE/GpSimd)
"""

import sys

try:
    import concourse.bass as bass
except ImportError:  # pragma: no cover - fallback when sitecustomize absent
    sys.path.insert(0, "/opt/trn_rl_repo")
    import concourse.bass as bass

import numpy as np
import ml_dtypes

import concourse.mybir as mybir
import concourse.tile as tile
from concourse import bacc
from concourse.bass import ts
from concourse.bass_utils import run_bass_kernel_spmd
from concourse.masks import make_identity

AF = mybir.ActivationFunctionType
ALU = mybir.AluOpType
F32 = mybir.dt.float32
BF16 = mybir.dt.bfloat16
BF16_NP = ml_dtypes.bfloat16

P = 128
D = 1024
H = 2048
E = D
ND = D // P            # 8 d-tiles
NH = H // P            # 16 h-tiles
CW = 256               # s-chunk width
M2 = CW // P           # 2 m-subtiles per chunk
EPS = 1e-5
INV_SQRT_D = 1.0 / 32.0


def build_nc(S=2048):
    NS = S // P            # s-tiles
    NCH = S // CW          # chunks

    nc = bacc.Bacc("TRN2", target_bir_lowering=False, debug=False, num_devices=8)

    x_res = nc.dram_tensor("x_res", [S, D], F32, kind="ExternalInput").ap()
    xT = nc.dram_tensor("xT", [D, S], BF16, kind="ExternalInput").ap()
    G = nc.dram_tensor("G", [D, D], BF16, kind="ExternalInput").ap()
    WvT = nc.dram_tensor("WvT", [D, E], BF16, kind="ExternalInput").ap()
    W1T = nc.dram_tensor("W1T", [D, H], BF16, kind="ExternalInput").ap()
    W2T = nc.dram_tensor("W2T", [H, D], BF16, kind="ExternalInput").ap()
    b1 = nc.dram_tensor("b1", [H], F32, kind="ExternalInput").ap()
    g1 = nc.dram_tensor("g1", [D], F32, kind="ExternalInput").ap()
    be1 = nc.dram_tensor("be1", [D], F32, kind="ExternalInput").ap()
    c1 = nc.dram_tensor("c1", [D], F32, kind="ExternalInput").ap()  # be1 + b2
    g2 = nc.dram_tensor("g2", [D], F32, kind="ExternalInput").ap()
    be2 = nc.dram_tensor("be2", [D], F32, kind="ExternalInput").ap()
    out = nc.dram_tensor("out", [S, D], F32, kind="ExternalOutput").ap()

    def bcast(ap_1d, n):
        return bass.AP(tensor=ap_1d.tensor, offset=ap_1d.offset, ap=[[0, P], [1, n]])

    def col(ap_1d, j):
        return ap_1d.rearrange("(a b) -> a b", b=1)[ts(j, P), :]

    with tile.TileContext(nc) as tc:
        with (
            tc.tile_pool(name="p256", bufs=2, space="PSUM") as p256,
            tc.tile_pool(name="p512", bufs=4, space="PSUM") as p512,
            tc.tile_pool(name="ptr", bufs=1, space="PSUM") as ptr,
            tc.tile_pool(name="pden", bufs=1, space="PSUM") as pden,
            tc.tile_pool(name="persist", bufs=1) as persist,
            tc.tile_pool(name="scal", bufs=24) as scal,
            tc.tile_pool(name="stats", bufs=8) as stats,
        ):
            # ---- constants ----
            ident = persist.tile([P, P], F32, tag="ident", name="ident")
            make_identity(nc, ident)
            ones_bf = persist.tile([P, 1], BF16, tag="ones", name="ones_bf")
            nc.vector.memset(ones_bf, 1.0)
            eps_t = persist.tile([P, 1], F32, tag="eps", name="eps_t")
            nc.vector.memset(eps_t, EPS)
            g1bc = persist.tile([P, D], BF16, tag="g1bc", name="g1bc")
            nc.gpsimd.dma_start(out=g1bc, in_=bcast(g1, D))
            c1bc = persist.tile([P, D], F32, tag="c1bc", name="c1bc")
            nc.gpsimd.dma_start(out=c1bc, in_=bcast(c1, D))
            g2bc = persist.tile([P, D], BF16, tag="g2bc", name="g2bc")
            nc.gpsimd.dma_start(out=g2bc, in_=bcast(g2, D))
            be2bc = persist.tile([P, D], BF16, tag="be2bc", name="be2bc")
            nc.gpsimd.dma_start(out=be2bc, in_=bcast(be2, D))
            b1col = []
            for n in range(NH):
                t = persist.tile([P, 1], F32, tag=f"b1c{n}", name=f"b1col{n}")
                nc.gpsimd.dma_start(out=t, in_=col(b1, n))
                b1col.append(t)
            g1col, be1col = [], []
            for j in range(ND):
                t = persist.tile([P, 1], F32, tag=f"g1c{j}", name=f"g1col{j}")
                nc.gpsimd.dma_start(out=t, in_=col(g1, j))
                g1col.append(t)
                t = persist.tile([P, 1], F32, tag=f"be1c{j}", name=f"be1col{j}")
                nc.gpsimd.dma_start(out=t, in_=col(be1, j))
                be1col.append(t)

            # ---- persistent tensors ----
            zT = [persist.tile([P, S], BF16, tag=f"zT{i}", name=f"zT{i}")
                  for i in range(ND)]
            v = [persist.tile([P, E], BF16, tag=f"v{t}", name=f"v{t}")
                 for t in range(NS)]
            # W1T packed resident: [p, j, h] with d = j*128+p
            w1p = persist.tile([P, ND, H], BF16, tag="w1p", name="w1p")

            # ===== phase 1: zT and v, streaming xT chunk-by-chunk so the
            # first matmul starts after ~2.5MB of DMA instead of ~12MB =====
            with (
                tc.tile_pool(name="ph1", bufs=1) as ph1,
                tc.tile_pool(name="xp1", bufs=20) as xp1,
            ):
                gp = ph1.tile([P, ND, D], BF16, tag="gp", name="gp")
                for j in range(ND):
                    nc.sync.dma_start(out=gp[:, j, :], in_=G[ts(j, P), :])
                # first zT group only needs gp + xs(0); wv queued after xs(0)
                # so the first matmuls start ~2MB of DMA earlier
                xs0 = []
                for j in range(ND):
                    t = xp1.tile([P, CW], BF16, tag="xs", name=f"xs0_{j}")
                    nc.sync.dma_start(out=t, in_=xT[ts(j, P), ts(0, CW)])
                    xs0.append(t)
                wv_sb = [ph1.tile([P, E], BF16, tag=f"wv{j}", name=f"wv{j}")
                         for j in range(ND)]
                for j in range(ND):
                    nc.sync.dma_start(out=wv_sb[j], in_=WvT[ts(j, P), :])

                for sc in range(S // CW):
                    if sc == 0:
                        xs = xs0
                    else:
                        xs = []
                        for j in range(ND):
                            t = xp1.tile([P, CW], BF16, tag="xs", name=f"xs{sc}_{j}")
                            nc.sync.dma_start(out=t, in_=xT[ts(j, P), ts(sc, CW)])
                            xs.append(t)
                    # zT[i][:, sc] = sum_j G[j-block, i-slice]^T @ xT[j][:, sc]
                    for i in range(ND):
                        ps = p256.tile([P, CW], F32, tag="mm", name=f"zps{i}_{sc}")
                        for j in range(ND):
                            nc.tensor.matmul(ps, gp[:, j, ts(i, P)], xs[j],
                                             start=(j == 0), stop=(j == ND - 1))
                        nc.vector.tensor_copy(out=zT[i][:, ts(sc, CW)], in_=ps)
                    # v[t][:, ec] = sum_j xT[j][:, t-slice]^T @ WvT[j][:, ec]
                    for tl in range(M2):
                        t_ = sc * M2 + tl
                        for ec in range(2):
                            ps = p512.tile([P, 512], F32, tag="mm",
                                           name=f"vps{t_}_{ec}")
                            for j in range(ND):
                                nc.tensor.matmul(ps, xs[j][:, ts(tl, P)],
                                                 wv_sb[j][:, ts(ec, 512)],
                                                 start=(j == 0), stop=(j == ND - 1))
                            nc.vector.tensor_copy(out=v[t_][:, ts(ec, 512)], in_=ps)
                # W1T needed from body 1 onwards; emit after the phase-1 DMAs
                for j in range(ND):
                    nc.sync.dma_start(out=w1p[:, j, :], in_=W1T[ts(j, P), :])

            # ======== chunk loop, software-pipelined: FFN runs one chunk
            # behind attention so the LN1->transpose dependency chain hides
            # under FFN(c-1)'s PE work (no PE stall, no HAM re-throttle) ====
            with (
                tc.tile_pool(name="xTc", bufs=9) as xTcp,
                tc.tile_pool(name="PT", bufs=17) as PTp,
                tc.tile_pool(name="hTc", bufs=18) as hTcp,
                tc.tile_pool(name="uT", bufs=17) as uTp,
                tc.tile_pool(name="w2s", bufs=6) as w2sp,
                tc.tile_pool(name="xmp", bufs=3) as xmp,
                tc.tile_pool(name="f32s", bufs=10) as f32sp,
            ):
                hc_prev = hr_prev = None
                for c in range(NCH + 1):
                    hp_cur, hr_cur = [], []
                    if c < NCH:
                        # ---- prefetches ----
                        xc = []
                        for j in range(ND):
                            t = xTcp.tile([P, CW], BF16, tag="xc", name=f"xc{c}_{j}")
                            nc.sync.dma_start(out=t, in_=xT[ts(j, P), ts(c, CW)])
                            xc.append(t)
                        xm = []
                        for m in range(M2):
                            t = xmp.tile([P, D], F32, tag="xm", name=f"xm{c}_{m}")
                            nc.sync.dma_start(out=t, in_=x_res[ts(c * M2 + m, P), :])
                            xm.append(t)
                        # ---- scoresT + exp ----
                        pt = []
                        for t_ in range(NS):
                            ps = p256.tile([P, CW], F32, tag="mm", name=f"sps{c}_{t_}")
                            for i in range(ND):
                                nc.tensor.matmul(ps, zT[i][:, ts(t_, P)], xc[i],
                                                 start=(i == 0), stop=(i == ND - 1))
                            t = PTp.tile([P, CW], BF16, tag="pt", name=f"pt{c}_{t_}")
                            nc.scalar.activation(out=t, in_=ps, func=AF.Exp,
                                                 scale=INV_SQRT_D)
                            pt.append(t)
                        # ---- PV + denom; normalize + residual + LN1 ----
                        for m in range(M2):
                            pa = [p512.tile([P, 512], F32, tag="mm",
                                            name=f"pa{c}_{m}_{ec}")
                                  for ec in range(2)]
                            pd = pden.tile([P, 1], F32, tag="den", name=f"pd{c}_{m}")
                            for t_ in range(NS):
                                lhs = pt[t_][:, ts(m, P)]
                                nc.tensor.matmul(pa[0], lhs, v[t_][:, 0:512],
                                                 start=(t_ == 0), stop=(t_ == NS - 1))
                                nc.tensor.matmul(pa[1], lhs, v[t_][:, 512:1024],
                                                 start=(t_ == 0), stop=(t_ == NS - 1))
                                nc.tensor.matmul(pd, lhs, ones_bf,
                                                 start=(t_ == 0), stop=(t_ == NS - 1))
                            r = scal.tile([P, 1], F32, tag="r", name=f"r{c}_{m}")
                            nc.vector.reciprocal(r, pd)
                            hp = f32sp.tile([P, D], F32, tag="f32", name=f"hp{c}_{m}")
                            for ec in range(2):
                                nc.vector.scalar_tensor_tensor(
                                    out=hp[:, ts(ec, 512)], in0=pa[ec], scalar=r,
                                    in1=xm[m][:, ts(ec, 512)],
                                    op0=ALU.mult, op1=ALU.add)
                            # LN1
                            st = stats.tile([P, 2, 6], F32, tag="st", name=f"st{c}_{m}")
                            for hf in range(2):
                                nc.vector.bn_stats(out=st[:, hf, :],
                                                   in_=hp[:, ts(hf, 512)])
                            mv = scal.tile([P, 2], F32, tag="mv", name=f"mv{c}_{m}")
                            nc.vector.bn_aggr(out=mv, in_=st)
                            rstd = scal.tile([P, 1], F32, tag="rstd",
                                             name=f"rstd{c}_{m}")
                            nc.scalar.activation(out=rstd, in_=mv[:, 1:2],
                                                 func=AF.Sqrt, bias=eps_t)
                            nc.vector.reciprocal(rstd, rstd)
                            nc.vector.tensor_scalar(out=hp, in0=hp,
                                                    scalar1=mv[:, 0:1], scalar2=rstd,
                                                    op0=ALU.subtract, op1=ALU.mult)
                            hp_cur.append(hp)
                            # natural-layout affine residual: g1*t1 + (be1+b2)
                            hrm = f32sp.tile([P, D], F32, tag="f32",
                                             name=f"hr{c}_{m}")
                            nc.gpsimd.tensor_mul(hrm, hp, g1bc)
                            nc.gpsimd.tensor_add(hrm, hrm, c1bc)
                            hr_cur.append(hrm)
                    if c > 0:
                        cp = c - 1
                        # w2n prefetch first: FFN1 provides ~14us of lead time
                        w2n = []
                        for n in range(NH):
                            t = w2sp.tile([P, D], BF16, tag="w2", name=f"w2_{cp}_{n}")
                            nc.sync.dma_start(out=t, in_=W2T[ts(n, P), :])
                            w2n.append(t)
                        # ---- FFN1(cp) ----
                        ut = []
                        for n in range(NH):
                            ps = p256.tile([P, CW], F32, tag="mm", name=f"ups{cp}_{n}")
                            for j in range(ND):
                                nc.tensor.matmul(ps, w1p[:, j, ts(n, P)], hc_prev[j],
                                                 start=(j == 0), stop=(j == ND - 1))
                            t = uTp.tile([P, CW], BF16, tag="ut", name=f"ut{cp}_{n}")
                            nc.scalar.activation(out=t, in_=ps, func=AF.Relu,
                                                 bias=b1col[n])
                            ut.append(t)
                        # ---- FFN2(cp): 4 concurrent psums (dc x m) ----
                        psm = [[p512.tile([P, 512], F32, tag="mm",
                                          name=f"fps{cp}_{dc}_{m}")
                                for m in range(M2)] for dc in range(2)]
                        for n in range(NH):
                            for dc in range(2):
                                for m in range(M2):
                                    nc.tensor.matmul(psm[dc][m], ut[n][:, ts(m, P)],
                                                     w2n[n][:, ts(dc, 512)],
                                                     start=(n == 0),
                                                     stop=(n == NH - 1))
                        u2 = [f32sp.tile([P, D], F32, tag="f32", name=f"u2{cp}_{m}")
                              for m in range(M2)]
                        for dc in range(2):
                            for m in range(M2):
                                nc.vector.scalar_tensor_tensor(
                                    out=u2[m][:, ts(dc, 512)], in0=psm[dc][m],
                                    scalar=1.0, in1=hr_prev[m][:, ts(dc, 512)],
                                    op0=ALU.mult, op1=ALU.add)
                        for m in range(M2):
                            sq = cp * M2 + m
                            st = stats.tile([P, 2, 6], F32, tag="st",
                                            name=f"st2{cp}_{m}")
                            for hf in range(2):
                                nc.vector.bn_stats(out=st[:, hf, :],
                                                   in_=u2[m][:, ts(hf, 512)])
                            mv = scal.tile([P, 2], F32, tag="mv", name=f"mv2{cp}_{m}")
                            nc.vector.bn_aggr(out=mv, in_=st)
                            rstd = scal.tile([P, 1], F32, tag="rstd",
                                             name=f"rstd2{cp}_{m}")
                            nc.scalar.activation(out=rstd, in_=mv[:, 1:2],
                                                 func=AF.Sqrt, bias=eps_t)
                            nc.vector.reciprocal(rstd, rstd)
                            nc.vector.tensor_scalar(out=u2[m], in0=u2[m],
                                                    scalar1=mv[:, 0:1], scalar2=rstd,
                                                    op0=ALU.subtract, op1=ALU.mult)
                            ot = f32sp.tile([P, D], F32, tag="f32", name=f"ot{cp}_{m}")
                            # last body: DVE for the tail affine (gpsimd is ~3x
                            # slower and would sit on the critical path)
                            eng = nc.vector if cp == NCH - 1 else nc.gpsimd
                            eng.tensor_mul(ot, u2[m], g2bc)
                            eng.tensor_add(ot, ot, be2bc)
                            nc.sync.dma_start(out=out[ts(sq, P), :], in_=ot)
                    if c < NCH:
                        # ---- hT transposes (LN1 affine folded into DVE copy);
                        # emitted after FFN(c-1) so the LN1 chain is hidden ----
                        hc = [hTcp.tile([P, CW], BF16, tag="hc", name=f"hc{c}_{j}")
                              for j in range(ND)]
                        for m in range(M2):
                            for j in range(ND):
                                pst = ptr.tile([P, P], F32, tag="tr",
                                               name=f"tr{c}_{m}_{j}")
                                nc.tensor.transpose(pst, hp_cur[m][:, ts(j, P)], ident)
                                if c == NCH - 1:
                                    # last chunk: DVE is busy with LN2(c-1);
                                    # use idle ScalarE so the tail FFN isn't
                                    # gated on these copies
                                    nc.scalar.activation(out=hc[j][:, ts(m, P)],
                                                         in_=pst, func=AF.Identity,
                                                         bias=be1col[j],
                                                         scale=g1col[j])
                                else:
                                    nc.vector.tensor_scalar(out=hc[j][:, ts(m, P)],
                                                            in0=pst,
                                                            scalar1=g1col[j],
                                                            scalar2=be1col[j],
                                                            op0=ALU.mult,
                                                            op1=ALU.add)
                        hc_prev, hr_prev = hc, hr_cur

    nc.compile()
    return nc


_CACHE = {}


def _get_nc(S):
    if S not in _CACHE:
        _CACHE[S] = build_nc(S)
    return _CACHE[S]


def kernel(x, Wq, Wk, Wv, W1, b1, W2, b2, g1, be1, g2, be2):
    x = np.asarray(x, np.float32)
    B, S, D_ = x.shape
    nc = _get_nc(S)

    def bft(a):  # transpose + cast to bf16, contiguous
        return np.ascontiguousarray(np.asarray(a, np.float32).T).astype(BF16_NP)

    Gm = (np.asarray(Wk, np.float32).T @ np.asarray(Wq, np.float32)).astype(BF16_NP)
    shared = {
        "G": Gm, "WvT": bft(Wv), "W1T": bft(W1), "W2T": bft(W2),
        "b1": np.asarray(b1, np.float32),
        "g1": np.asarray(g1, np.float32),
        "be1": np.asarray(be1, np.float32),
        "c1": np.asarray(be1, np.float32) + np.asarray(b2, np.float32),
        "g2": np.asarray(g2, np.float32),
        "be2": np.asarray(be2, np.float32),
    }
    in_maps = []
    for b in range(B):
        m = dict(shared)
        m["x_res"] = np.ascontiguousarray(x[b])
        m["xT"] = bft(x[b])
        in_maps.append(m)

    res = run_bass_kernel_spmd(nc, in_maps, core_ids=list(range(B)))
    return np.stack([np.asarray(res.results[b]["out"], np.float32)
                     for b in range(B)], axis=0)


# revision 25
# speedup vs baseline: 1.0837x; 1.0148x over previous
"""MiniTransformer layer on 8 TRN2 NeuronCores.

Strategy: data-parallel over batch (B=8 -> one batch element per core, no
collectives). Per core, one full transformer block over [S=2048, D=1024].

All matmuls run in bf16 (fp32 PSUM accumulation). Host-side layout prep:
weights pre-transposed/cast to bf16; the q/k projections are folded into a
single matrix G = Wk^T @ Wq so scoresT = (x@G) @ x^T (saves one projection
and keeps both score operands in transposed layout -> softmax needs no
on-chip transposes; max-subtraction is skipped since |scores/sqrt(D)| < 3).

Per-core flow:
  phase1:  zT[d',s] = G^T-contraction over xT;  v[s,e] = x @ Wv^T
  chunks of 256 tokens:
    scoresT[sk, sq] = zT^T-tiles @ xT-chunk     (PSUM f32)
    PT = exp(scoresT/32) bf16                   (ScalarE, no max needed)
    attn = PT^T @ v ; den = PT^T @ ones         (PE, accumulated over sk)
    h = attn/den + x ; LN1                      (DVE)
    hT via PE transpose; LN1 affine folded into the PSUM->SBUF copy (DVE)
    uT = W1T^T @ hT ; relu(+b1)                 (PE + ScalarE)
    ff = uT^T @ W2T ; + (g1*t1+be1+b2) ; LN2    (PE + DVE/GpSimd)
"""

import sys

try:
    import concourse.bass as bass
except ImportError:  # pragma: no cover - fallback when sitecustomize absent
    sys.path.insert(0, "/opt/trn_rl_repo")
    import concourse.bass as bass

import numpy as np
import ml_dtypes

import concourse.mybir as mybir
import concourse.tile as tile
from concourse import bacc
from concourse.bass import ts
from concourse.bass_utils import run_bass_kernel_spmd
from concourse.masks import make_identity

AF = mybir.ActivationFunctionType
ALU = mybir.AluOpType
F32 = mybir.dt.float32
BF16 = mybir.dt.bfloat16
BF16_NP = ml_dtypes.bfloat16

P = 128
D = 1024
H = 2048
E = D
ND = D // P            # 8 d-tiles
NH = H // P            # 16 h-tiles
CW = 256               # s-chunk width
M2 = CW // P           # 2 m-subtiles per chunk
EPS = 1e-5
INV_SQRT_D = 1.0 / 32.0


def build_nc(S=2048):
    NS = S // P            # s-tiles
    NCH = S // CW          # chunks

    nc = bacc.Bacc("TRN2", target_bir_lowering=False, debug=False, num_devices=8)

    x_res = nc.dram_tensor("x_res", [S, D], F32, kind="ExternalInput").ap()
    xT = nc.dram_tensor("xT", [D, S], BF16, kind="ExternalInput").ap()
    G = nc.dram_tensor("G", [D, D], BF16, kind="ExternalInput").ap()
    WvT = nc.dram_tensor("WvT", [D, E], BF16, kind="ExternalInput").ap()
    W1T = nc.dram_tensor("W1T", [D, H], BF16, kind="ExternalInput").ap()
    W2T = nc.dram_tensor("W2T", [H, D], BF16, kind="ExternalInput").ap()
    b1 = nc.dram_tensor("b1", [H], F32, kind="ExternalInput").ap()
    g1 = nc.dram_tensor("g1", [D], F32, kind="ExternalInput").ap()
    be1 = nc.dram_tensor("be1", [D], F32, kind="ExternalInput").ap()
    c1 = nc.dram_tensor("c1", [D], F32, kind="ExternalInput").ap()  # be1 + b2
    g2 = nc.dram_tensor("g2", [D], F32, kind="ExternalInput").ap()
    be2 = nc.dram_tensor("be2", [D], F32, kind="ExternalInput").ap()
    out = nc.dram_tensor("out", [S, D], F32, kind="ExternalOutput").ap()

    def bcast(ap_1d, n):
        return bass.AP(tensor=ap_1d.tensor, offset=ap_1d.offset, ap=[[0, P], [1, n]])

    def col(ap_1d, j):
        return ap_1d.rearrange("(a b) -> a b", b=1)[ts(j, P), :]

    with tile.TileContext(nc) as tc:
        with (
            tc.tile_pool(name="p256", bufs=3, space="PSUM") as p256,
            tc.tile_pool(name="p512", bufs=4, space="PSUM") as p512,
            tc.tile_pool(name="ptr", bufs=1, space="PSUM") as ptr,
            tc.tile_pool(name="persist", bufs=1) as persist,
            tc.tile_pool(name="scal", bufs=24) as scal,
            tc.tile_pool(name="stats", bufs=8) as stats,
        ):
            # ---- constants ----
            ident = persist.tile([P, P], F32, tag="ident", name="ident")
            make_identity(nc, ident)
            ones_bf = persist.tile([P, 1], BF16, tag="ones", name="ones_bf")
            nc.vector.memset(ones_bf, 1.0)
            eps_t = persist.tile([P, 1], F32, tag="eps", name="eps_t")
            nc.vector.memset(eps_t, EPS)
            g1bc = persist.tile([P, D], BF16, tag="g1bc", name="g1bc")
            nc.gpsimd.dma_start(out=g1bc, in_=bcast(g1, D))
            c1bc = persist.tile([P, D], F32, tag="c1bc", name="c1bc")
            nc.gpsimd.dma_start(out=c1bc, in_=bcast(c1, D))
            g2bc = persist.tile([P, D], BF16, tag="g2bc", name="g2bc")
            nc.gpsimd.dma_start(out=g2bc, in_=bcast(g2, D))
            be2bc = persist.tile([P, D], BF16, tag="be2bc", name="be2bc")
            nc.gpsimd.dma_start(out=be2bc, in_=bcast(be2, D))
            b1col = []
            for n in range(NH):
                t = persist.tile([P, 1], F32, tag=f"b1c{n}", name=f"b1col{n}")
                nc.gpsimd.dma_start(out=t, in_=col(b1, n))
                b1col.append(t)
            g1col, be1col = [], []
            for j in range(ND):
                t = persist.tile([P, 1], F32, tag=f"g1c{j}", name=f"g1col{j}")
                nc.gpsimd.dma_start(out=t, in_=col(g1, j))
                g1col.append(t)
                t = persist.tile([P, 1], F32, tag=f"be1c{j}", name=f"be1col{j}")
                nc.gpsimd.dma_start(out=t, in_=col(be1, j))
                be1col.append(t)

            # ---- persistent tensors ----
            zT = [persist.tile([P, S], BF16, tag=f"zT{i}", name=f"zT{i}")
                  for i in range(ND)]
            v = [persist.tile([P, E], BF16, tag=f"v{t}", name=f"v{t}")
                 for t in range(NS)]
            # W1T packed resident: [p, j, h] with d = j*128+p
            w1p = persist.tile([P, ND, H], BF16, tag="w1p", name="w1p")

            # ===== phase 1: zT and v, streaming xT chunk-by-chunk so the
            # first matmul starts after ~2.5MB of DMA instead of ~12MB =====
            with (
                tc.tile_pool(name="ph1", bufs=1) as ph1,
                tc.tile_pool(name="xp1", bufs=20) as xp1,
            ):
                gp = ph1.tile([P, ND, D], BF16, tag="gp", name="gp")
                for j in range(ND):
                    nc.sync.dma_start(out=gp[:, j, :], in_=G[ts(j, P), :])
                # first zT group only needs gp + xs(0); wv queued after xs(0)
                # so the first matmuls start ~2MB of DMA earlier
                xs0 = []
                for j in range(ND):
                    t = xp1.tile([P, CW], BF16, tag="xs", name=f"xs0_{j}")
                    nc.sync.dma_start(out=t, in_=xT[ts(j, P), ts(0, CW)])
                    xs0.append(t)
                wv_sb = [ph1.tile([P, E], BF16, tag=f"wv{j}", name=f"wv{j}")
                         for j in range(ND)]
                for ec in range(2):
                    for j in range(ND):
                        nc.sync.dma_start(out=wv_sb[j][:, ts(ec, 512)],
                                          in_=WvT[ts(j, P), ts(ec, 512)])

                for sc in range(S // CW):
                    if sc == 0:
                        xs = xs0
                    else:
                        xs = []
                        for j in range(ND):
                            t = xp1.tile([P, CW], BF16, tag="xs", name=f"xs{sc}_{j}")
                            nc.sync.dma_start(out=t, in_=xT[ts(j, P), ts(sc, CW)])
                            xs.append(t)
                    # zT[i][:, sc] = sum_j G[j-block, i-slice]^T @ xT[j][:, sc]
                    for i in range(ND):
                        ps = p256.tile([P, CW], F32, tag="mm", name=f"zps{i}_{sc}")
                        for j in range(ND):
                            nc.tensor.matmul(ps, gp[:, j, ts(i, P)], xs[j],
                                             start=(j == 0), stop=(j == ND - 1))
                        nc.vector.tensor_copy(out=zT[i][:, ts(sc, CW)], in_=ps)
                    # v[t][:, ec] = sum_j xT[j][:, t-slice]^T @ WvT[j][:, ec]
                    for tl in range(M2):
                        t_ = sc * M2 + tl
                        for ec in range(2):
                            ps = p512.tile([P, 512], F32, tag="mm",
                                           name=f"vps{t_}_{ec}")
                            for j in range(ND):
                                nc.tensor.matmul(ps, xs[j][:, ts(tl, P)],
                                                 wv_sb[j][:, ts(ec, 512)],
                                                 start=(j == 0), stop=(j == ND - 1))
                            nc.vector.tensor_copy(out=v[t_][:, ts(ec, 512)], in_=ps)
                # W1T needed from body 1 onwards; emit after the phase-1 DMAs
                for j in range(ND):
                    nc.sync.dma_start(out=w1p[:, j, :], in_=W1T[ts(j, P), :])

            # ======== chunk loop, software-pipelined: FFN runs one chunk
            # behind attention so the LN1->transpose dependency chain hides
            # under FFN(c-1)'s PE work (no PE stall, no HAM re-throttle) ====
            with (
                tc.tile_pool(name="xTc", bufs=9) as xTcp,
                tc.tile_pool(name="PT", bufs=17) as PTp,
                tc.tile_pool(name="hTc", bufs=18) as hTcp,
                tc.tile_pool(name="uT", bufs=17) as uTp,
                tc.tile_pool(name="w2s", bufs=6) as w2sp,
                tc.tile_pool(name="xmp", bufs=3) as xmp,
                tc.tile_pool(name="f32s", bufs=10) as f32sp,
            ):
                hc_prev = hr_prev = None
                for c in range(NCH + 1):
                    hp_cur, hr_cur = [], []
                    if c < NCH:
                        # ---- prefetches ----
                        xc = []
                        for j in range(ND):
                            t = xTcp.tile([P, CW], BF16, tag="xc", name=f"xc{c}_{j}")
                            nc.sync.dma_start(out=t, in_=xT[ts(j, P), ts(c, CW)])
                            xc.append(t)
                        xm = []
                        for m in range(M2):
                            t = xmp.tile([P, D], F32, tag="xm", name=f"xm{c}_{m}")
                            nc.sync.dma_start(out=t, in_=x_res[ts(c * M2 + m, P), :])
                            xm.append(t)
                        # ---- scoresT + exp ----
                        pt = []
                        for t_ in range(NS):
                            ps = p256.tile([P, CW], F32, tag="mm", name=f"sps{c}_{t_}")
                            for i in range(ND):
                                nc.tensor.matmul(ps, zT[i][:, ts(t_, P)], xc[i],
                                                 start=(i == 0), stop=(i == ND - 1))
                            t = PTp.tile([P, CW], BF16, tag="pt", name=f"pt{c}_{t_}")
                            nc.scalar.activation(out=t, in_=ps, func=AF.Exp,
                                                 scale=INV_SQRT_D)
                            pt.append(t)
                        # ---- PV + denom; normalize + residual + LN1 ----
                        for m in range(M2):
                            pa = [p512.tile([P, 512], F32, tag="mm",
                                            name=f"pa{c}_{m}_{ec}")
                                  for ec in range(2)]
                            pd = ptr.tile([P, 1], F32, tag="tr", name=f"pd{c}_{m}")
                            for t_ in range(NS):
                                lhs = pt[t_][:, ts(m, P)]
                                nc.tensor.matmul(pa[0], lhs, v[t_][:, 0:512],
                                                 start=(t_ == 0), stop=(t_ == NS - 1))
                                nc.tensor.matmul(pa[1], lhs, v[t_][:, 512:1024],
                                                 start=(t_ == 0), stop=(t_ == NS - 1))
                                nc.tensor.matmul(pd, lhs, ones_bf,
                                                 start=(t_ == 0), stop=(t_ == NS - 1))
                            r = scal.tile([P, 1], F32, tag="r", name=f"r{c}_{m}")
                            nc.vector.reciprocal(r, pd)
                            hp = f32sp.tile([P, D], F32, tag="f32", name=f"hp{c}_{m}")
                            for ec in range(2):
                                nc.vector.scalar_tensor_tensor(
                                    out=hp[:, ts(ec, 512)], in0=pa[ec], scalar=r,
                                    in1=xm[m][:, ts(ec, 512)],
                                    op0=ALU.mult, op1=ALU.add)
                            # LN1
                            st = stats.tile([P, 2, 6], F32, tag="st", name=f"st{c}_{m}")
                            for hf in range(2):
                                nc.vector.bn_stats(out=st[:, hf, :],
                                                   in_=hp[:, ts(hf, 512)])
                            mv = scal.tile([P, 2], F32, tag="mv", name=f"mv{c}_{m}")
                            nc.vector.bn_aggr(out=mv, in_=st)
                            rstd = scal.tile([P, 1], F32, tag="rstd",
                                             name=f"rstd{c}_{m}")
                            nc.scalar.activation(out=rstd, in_=mv[:, 1:2],
                                                 func=AF.Sqrt, bias=eps_t)
                            nc.vector.reciprocal(rstd, rstd)
                            nc.vector.tensor_scalar(out=hp, in0=hp,
                                                    scalar1=mv[:, 0:1], scalar2=rstd,
                                                    op0=ALU.subtract, op1=ALU.mult)
                            hp_cur.append(hp)
                            # natural-layout affine residual: g1*t1 + (be1+b2)
                            hrm = f32sp.tile([P, D], F32, tag="f32",
                                             name=f"hr{c}_{m}")
                            nc.gpsimd.tensor_mul(hrm, hp, g1bc)
                            nc.gpsimd.tensor_add(hrm, hrm, c1bc)
                            hr_cur.append(hrm)
                    if c > 0:
                        cp = c - 1
                        # w2n prefetch first: FFN1 provides ~14us of lead time
                        w2n = []
                        for n in range(NH):
                            t = w2sp.tile([P, D], BF16, tag="w2", name=f"w2_{cp}_{n}")
                            nc.sync.dma_start(out=t, in_=W2T[ts(n, P), :])
                            w2n.append(t)
                        # ---- FFN1(cp) ----
                        ut = []
                        for n in range(NH):
                            ps = p256.tile([P, CW], F32, tag="mm", name=f"ups{cp}_{n}")
                            for j in range(ND):
                                nc.tensor.matmul(ps, w1p[:, j, ts(n, P)], hc_prev[j],
                                                 start=(j == 0), stop=(j == ND - 1))
                            t = uTp.tile([P, CW], BF16, tag="ut", name=f"ut{cp}_{n}")
                            nc.scalar.activation(out=t, in_=ps, func=AF.Relu,
                                                 bias=b1col[n])
                            ut.append(t)
                        # ---- FFN2(cp): 4 concurrent psums (dc x m) ----
                        psm = [[p512.tile([P, 512], F32, tag="mm",
                                          name=f"fps{cp}_{dc}_{m}")
                                for m in range(M2)] for dc in range(2)]
                        for n in range(NH):
                            for dc in range(2):
                                for m in range(M2):
                                    nc.tensor.matmul(psm[dc][m], ut[n][:, ts(m, P)],
                                                     w2n[n][:, ts(dc, 512)],
                                                     start=(n == 0),
                                                     stop=(n == NH - 1))
                        u2 = [f32sp.tile([P, D], F32, tag="f32", name=f"u2{cp}_{m}")
                              for m in range(M2)]
                        for dc in range(2):
                            for m in range(M2):
                                nc.vector.scalar_tensor_tensor(
                                    out=u2[m][:, ts(dc, 512)], in0=psm[dc][m],
                                    scalar=1.0, in1=hr_prev[m][:, ts(dc, 512)],
                                    op0=ALU.mult, op1=ALU.add)
                        for m in range(M2):
                            sq = cp * M2 + m
                            st = stats.tile([P, 2, 6], F32, tag="st",
                                            name=f"st2{cp}_{m}")
                            for hf in range(2):
                                nc.vector.bn_stats(out=st[:, hf, :],
                                                   in_=u2[m][:, ts(hf, 512)])
                            mv = scal.tile([P, 2], F32, tag="mv", name=f"mv2{cp}_{m}")
                            nc.vector.bn_aggr(out=mv, in_=st)
                            rstd = scal.tile([P, 1], F32, tag="rstd",
                                             name=f"rstd2{cp}_{m}")
                            nc.scalar.activation(out=rstd, in_=mv[:, 1:2],
                                                 func=AF.Sqrt, bias=eps_t)
                            nc.vector.reciprocal(rstd, rstd)
                            nc.vector.tensor_scalar(out=u2[m], in0=u2[m],
                                                    scalar1=mv[:, 0:1], scalar2=rstd,
                                                    op0=ALU.subtract, op1=ALU.mult)
                            ot = f32sp.tile([P, D], F32, tag="f32", name=f"ot{cp}_{m}")
                            # last body: DVE for the tail affine (gpsimd is ~3x
                            # slower and would sit on the critical path)
                            eng = nc.vector if cp == NCH - 1 else nc.gpsimd
                            eng.tensor_mul(ot, u2[m], g2bc)
                            eng.tensor_add(ot, ot, be2bc)
                            nc.sync.dma_start(out=out[ts(sq, P), :], in_=ot)
                    if c < NCH:
                        # ---- hT transposes (LN1 affine folded into DVE copy);
                        # emitted after FFN(c-1) so the LN1 chain is hidden ----
                        hc = [hTcp.tile([P, CW], BF16, tag="hc", name=f"hc{c}_{j}")
                              for j in range(ND)]
                        for m in range(M2):
                            for j in range(ND):
                                pst = ptr.tile([P, P], F32, tag="tr",
                                               name=f"tr{c}_{m}_{j}")
                                nc.tensor.transpose(pst, hp_cur[m][:, ts(j, P)], ident)
                                if c == NCH - 1:
                                    # last chunk: DVE is busy with LN2(c-1);
                                    # use idle ScalarE so the tail FFN isn't
                                    # gated on these copies
                                    nc.scalar.activation(out=hc[j][:, ts(m, P)],
                                                         in_=pst, func=AF.Identity,
                                                         bias=be1col[j],
                                                         scale=g1col[j])
                                else:
                                    nc.vector.tensor_scalar(out=hc[j][:, ts(m, P)],
                                                            in0=pst,
                                                            scalar1=g1col[j],
                                                            scalar2=be1col[j],
                                                            op0=ALU.mult,
                                                            op1=ALU.add)
                        hc_prev, hr_prev = hc, hr_cur

    nc.compile()
    return nc


_CACHE = {}


def _get_nc(S):
    if S not in _CACHE:
        _CACHE[S] = build_nc(S)
    return _CACHE[S]


def kernel(x, Wq, Wk, Wv, W1, b1, W2, b2, g1, be1, g2, be2):
    x = np.asarray(x, np.float32)
    B, S, D_ = x.shape
    nc = _get_nc(S)

    def bft(a):  # transpose + cast to bf16, contiguous
        return np.ascontiguousarray(np.asarray(a, np.float32).T).astype(BF16_NP)

    Gm = (np.asarray(Wk, np.float32).T @ np.asarray(Wq, np.float32)).astype(BF16_NP)
    shared = {
        "G": Gm, "WvT": bft(Wv), "W1T": bft(W1), "W2T": bft(W2),
        "b1": np.asarray(b1, np.float32),
        "g1": np.asarray(g1, np.float32),
        "be1": np.asarray(be1, np.float32),
        "c1": np.asarray(be1, np.float32) + np.asarray(b2, np.float32),
        "g2": np.asarray(g2, np.float32),
        "be2": np.asarray(be2, np.float32),
    }
    in_maps = []
    for b in range(B):
        m = dict(shared)
        m["x_res"] = np.ascontiguousarray(x[b])
        m["xT"] = bft(x[b])
        in_maps.append(m)

    res = run_bass_kernel_spmd(nc, in_maps, core_ids=list(range(B)))
    return np.stack([np.asarray(res.results[b]["out"], np.float32)
                     for b in range(B)], axis=0)


# revision 26
# speedup vs baseline: 1.0838x; 1.0001x over previous
"""MiniTransformer layer on 8 TRN2 NeuronCores.

Strategy: data-parallel over batch (B=8 -> one batch element per core, no
collectives). Per core, one full transformer block over [S=2048, D=1024].

All matmuls run in bf16 (fp32 PSUM accumulation). Host-side layout prep:
weights pre-transposed/cast to bf16; the q/k projections are folded into a
single matrix G = Wk^T @ Wq so scoresT = (x@G) @ x^T (saves one projection
and keeps both score operands in transposed layout -> softmax needs no
on-chip transposes; max-subtraction is skipped since |scores/sqrt(D)| < 3).

Per-core flow:
  phase1:  zT[d',s] = G^T-contraction over xT;  v[s,e] = x @ Wv^T
  chunks of 256 tokens:
    scoresT[sk, sq] = zT^T-tiles @ xT-chunk     (PSUM f32)
    PT = exp(scoresT/32) bf16                   (ScalarE, no max needed)
    attn = PT^T @ v ; den = PT^T @ ones         (PE, accumulated over sk)
    h = attn/den + x ; LN1                      (DVE)
    hT via PE transpose; LN1 affine folded into the PSUM->SBUF copy (DVE)
    uT = W1T^T @ hT ; relu(+b1)                 (PE + ScalarE)
    ff = uT^T @ W2T ; + (g1*t1+be1+b2) ; LN2    (PE + DVE/GpSimd)
"""

import sys

try:
    import concourse.bass as bass
except ImportError:  # pragma: no cover - fallback when sitecustomize absent
    sys.path.insert(0, "/opt/trn_rl_repo")
    import concourse.bass as bass

import numpy as np
import ml_dtypes

import concourse.mybir as mybir
import concourse.tile as tile
from concourse import bacc
from concourse.bass import ts
from concourse.bass_utils import run_bass_kernel_spmd
from concourse.masks import make_identity

AF = mybir.ActivationFunctionType
ALU = mybir.AluOpType
F32 = mybir.dt.float32
BF16 = mybir.dt.bfloat16
BF16_NP = ml_dtypes.bfloat16

P = 128
D = 1024
H = 2048
E = D
ND = D // P            # 8 d-tiles
NH = H // P            # 16 h-tiles
CW = 256               # s-chunk width
M2 = CW // P           # 2 m-subtiles per chunk
EPS = 1e-5
INV_SQRT_D = 1.0 / 32.0


def build_nc(S=2048):
    NS = S // P            # s-tiles
    NCH = S // CW          # chunks

    nc = bacc.Bacc("TRN2", target_bir_lowering=False, debug=False, num_devices=8)

    x_res = nc.dram_tensor("x_res", [S, D], F32, kind="ExternalInput").ap()
    xT = nc.dram_tensor("xT", [D, S], BF16, kind="ExternalInput").ap()
    G = nc.dram_tensor("G", [D, D], BF16, kind="ExternalInput").ap()
    WvT = nc.dram_tensor("WvT", [D, E], BF16, kind="ExternalInput").ap()
    W1T = nc.dram_tensor("W1T", [D, H], BF16, kind="ExternalInput").ap()
    W2T = nc.dram_tensor("W2T", [H, D], BF16, kind="ExternalInput").ap()
    b1 = nc.dram_tensor("b1", [H], F32, kind="ExternalInput").ap()
    g1 = nc.dram_tensor("g1", [D], F32, kind="ExternalInput").ap()
    be1 = nc.dram_tensor("be1", [D], F32, kind="ExternalInput").ap()
    c1 = nc.dram_tensor("c1", [D], F32, kind="ExternalInput").ap()  # be1 + b2
    g2 = nc.dram_tensor("g2", [D], F32, kind="ExternalInput").ap()
    be2 = nc.dram_tensor("be2", [D], F32, kind="ExternalInput").ap()
    out = nc.dram_tensor("out", [S, D], F32, kind="ExternalOutput").ap()

    def bcast(ap_1d, n):
        return bass.AP(tensor=ap_1d.tensor, offset=ap_1d.offset, ap=[[0, P], [1, n]])

    def col(ap_1d, j):
        return ap_1d.rearrange("(a b) -> a b", b=1)[ts(j, P), :]

    with tile.TileContext(nc) as tc:
        with (
            tc.tile_pool(name="p256", bufs=3, space="PSUM") as p256,
            tc.tile_pool(name="p512", bufs=4, space="PSUM") as p512,
            tc.tile_pool(name="ptr", bufs=1, space="PSUM") as ptr,
            tc.tile_pool(name="persist", bufs=1) as persist,
            tc.tile_pool(name="scal", bufs=24) as scal,
            tc.tile_pool(name="stats", bufs=8) as stats,
        ):
            # ---- constants ----
            ident = persist.tile([P, P], F32, tag="ident", name="ident")
            make_identity(nc, ident)
            ones_bf = persist.tile([P, 1], BF16, tag="ones", name="ones_bf")
            nc.vector.memset(ones_bf, 1.0)
            eps_t = persist.tile([P, 1], F32, tag="eps", name="eps_t")
            nc.vector.memset(eps_t, EPS)
            g1bc = persist.tile([P, D], BF16, tag="g1bc", name="g1bc")
            nc.gpsimd.dma_start(out=g1bc, in_=bcast(g1, D))
            c1bc = persist.tile([P, D], F32, tag="c1bc", name="c1bc")
            nc.gpsimd.dma_start(out=c1bc, in_=bcast(c1, D))
            g2bc = persist.tile([P, D], BF16, tag="g2bc", name="g2bc")
            nc.gpsimd.dma_start(out=g2bc, in_=bcast(g2, D))
            be2bc = persist.tile([P, D], BF16, tag="be2bc", name="be2bc")
            nc.gpsimd.dma_start(out=be2bc, in_=bcast(be2, D))
            b1col = []
            for n in range(NH):
                t = persist.tile([P, 1], F32, tag=f"b1c{n}", name=f"b1col{n}")
                nc.gpsimd.dma_start(out=t, in_=col(b1, n))
                b1col.append(t)
            g1col, be1col = [], []
            for j in range(ND):
                t = persist.tile([P, 1], F32, tag=f"g1c{j}", name=f"g1col{j}")
                nc.gpsimd.dma_start(out=t, in_=col(g1, j))
                g1col.append(t)
                t = persist.tile([P, 1], F32, tag=f"be1c{j}", name=f"be1col{j}")
                nc.gpsimd.dma_start(out=t, in_=col(be1, j))
                be1col.append(t)

            # ---- persistent tensors ----
            zT = [persist.tile([P, S], BF16, tag=f"zT{i}", name=f"zT{i}")
                  for i in range(ND)]
            v = [persist.tile([P, E], BF16, tag=f"v{t}", name=f"v{t}")
                 for t in range(NS)]
            # W1T packed resident: [p, j, h] with d = j*128+p
            w1p = persist.tile([P, ND, H], BF16, tag="w1p", name="w1p")

            # ===== phase 1: zT and v, streaming xT chunk-by-chunk so the
            # first matmul starts after ~2.5MB of DMA instead of ~12MB =====
            with (
                tc.tile_pool(name="ph1", bufs=1) as ph1,
                tc.tile_pool(name="xp1", bufs=24) as xp1,
            ):
                gp = ph1.tile([P, ND, D], BF16, tag="gp", name="gp")
                for j in range(ND):
                    nc.sync.dma_start(out=gp[:, j, :], in_=G[ts(j, P), :])
                # first zT group only needs gp + xs(0); wv queued after xs(0)
                # so the first matmuls start ~2MB of DMA earlier
                xs0 = []
                for j in range(ND):
                    t = xp1.tile([P, CW], BF16, tag="xs", name=f"xs0_{j}")
                    nc.sync.dma_start(out=t, in_=xT[ts(j, P), ts(0, CW)])
                    xs0.append(t)
                xs1 = []
                for j in range(ND):
                    t = xp1.tile([P, CW], BF16, tag="xs", name=f"xs1_{j}")
                    nc.sync.dma_start(out=t, in_=xT[ts(j, P), ts(1, CW)])
                    xs1.append(t)
                wv_sb = [ph1.tile([P, E], BF16, tag=f"wv{j}", name=f"wv{j}")
                         for j in range(ND)]
                for ec in range(2):
                    for j in range(ND):
                        nc.sync.dma_start(out=wv_sb[j][:, ts(ec, 512)],
                                          in_=WvT[ts(j, P), ts(ec, 512)])

                for sc in range(S // CW):
                    if sc == 0:
                        xs = xs0
                    elif sc == 1:
                        xs = xs1
                    else:
                        xs = []
                        for j in range(ND):
                            t = xp1.tile([P, CW], BF16, tag="xs", name=f"xs{sc}_{j}")
                            nc.sync.dma_start(out=t, in_=xT[ts(j, P), ts(sc, CW)])
                            xs.append(t)
                    # zT[i][:, sc] = sum_j G[j-block, i-slice]^T @ xT[j][:, sc]
                    for i in range(ND):
                        ps = p256.tile([P, CW], F32, tag="mm", name=f"zps{i}_{sc}")
                        for j in range(ND):
                            nc.tensor.matmul(ps, gp[:, j, ts(i, P)], xs[j],
                                             start=(j == 0), stop=(j == ND - 1))
                        nc.vector.tensor_copy(out=zT[i][:, ts(sc, CW)], in_=ps)
                    # v[t][:, ec] = sum_j xT[j][:, t-slice]^T @ WvT[j][:, ec]
                    for tl in range(M2):
                        t_ = sc * M2 + tl
                        for ec in range(2):
                            ps = p512.tile([P, 512], F32, tag="mm",
                                           name=f"vps{t_}_{ec}")
                            for j in range(ND):
                                nc.tensor.matmul(ps, xs[j][:, ts(tl, P)],
                                                 wv_sb[j][:, ts(ec, 512)],
                                                 start=(j == 0), stop=(j == ND - 1))
                            nc.vector.tensor_copy(out=v[t_][:, ts(ec, 512)], in_=ps)
                # W1T needed from body 1 onwards; emit after the phase-1 DMAs
                for j in range(ND):
                    nc.sync.dma_start(out=w1p[:, j, :], in_=W1T[ts(j, P), :])

            # ======== chunk loop, software-pipelined: FFN runs one chunk
            # behind attention so the LN1->transpose dependency chain hides
            # under FFN(c-1)'s PE work (no PE stall, no HAM re-throttle) ====
            with (
                tc.tile_pool(name="xTc", bufs=9) as xTcp,
                tc.tile_pool(name="PT", bufs=17) as PTp,
                tc.tile_pool(name="hTc", bufs=18) as hTcp,
                tc.tile_pool(name="uT", bufs=17) as uTp,
                tc.tile_pool(name="w2s", bufs=6) as w2sp,
                tc.tile_pool(name="xmp", bufs=3) as xmp,
                tc.tile_pool(name="f32s", bufs=10) as f32sp,
            ):
                hc_prev = hr_prev = None
                for c in range(NCH + 1):
                    hp_cur, hr_cur = [], []
                    if c < NCH:
                        # ---- prefetches ----
                        xc = []
                        for j in range(ND):
                            t = xTcp.tile([P, CW], BF16, tag="xc", name=f"xc{c}_{j}")
                            nc.sync.dma_start(out=t, in_=xT[ts(j, P), ts(c, CW)])
                            xc.append(t)
                        xm = []
                        for m in range(M2):
                            t = xmp.tile([P, D], F32, tag="xm", name=f"xm{c}_{m}")
                            nc.sync.dma_start(out=t, in_=x_res[ts(c * M2 + m, P), :])
                            xm.append(t)
                        # ---- scoresT + exp ----
                        pt = []
                        for t_ in range(NS):
                            ps = p256.tile([P, CW], F32, tag="mm", name=f"sps{c}_{t_}")
                            for i in range(ND):
                                nc.tensor.matmul(ps, zT[i][:, ts(t_, P)], xc[i],
                                                 start=(i == 0), stop=(i == ND - 1))
                            t = PTp.tile([P, CW], BF16, tag="pt", name=f"pt{c}_{t_}")
                            nc.scalar.activation(out=t, in_=ps, func=AF.Exp,
                                                 scale=INV_SQRT_D)
                            pt.append(t)
                        # ---- PV + denom; normalize + residual + LN1 ----
                        for m in range(M2):
                            pa = [p512.tile([P, 512], F32, tag="mm",
                                            name=f"pa{c}_{m}_{ec}")
                                  for ec in range(2)]
                            pd = ptr.tile([P, 1], F32, tag="tr", name=f"pd{c}_{m}")
                            for t_ in range(NS):
                                lhs = pt[t_][:, ts(m, P)]
                                nc.tensor.matmul(pa[0], lhs, v[t_][:, 0:512],
                                                 start=(t_ == 0), stop=(t_ == NS - 1))
                                nc.tensor.matmul(pa[1], lhs, v[t_][:, 512:1024],
                                                 start=(t_ == 0), stop=(t_ == NS - 1))
                                nc.tensor.matmul(pd, lhs, ones_bf,
                                                 start=(t_ == 0), stop=(t_ == NS - 1))
                            r = scal.tile([P, 1], F32, tag="r", name=f"r{c}_{m}")
                            nc.vector.reciprocal(r, pd)
                            hp = f32sp.tile([P, D], F32, tag="f32", name=f"hp{c}_{m}")
                            for ec in range(2):
                                nc.vector.scalar_tensor_tensor(
                                    out=hp[:, ts(ec, 512)], in0=pa[ec], scalar=r,
                                    in1=xm[m][:, ts(ec, 512)],
                                    op0=ALU.mult, op1=ALU.add)
                            # LN1
                            st = stats.tile([P, 2, 6], F32, tag="st", name=f"st{c}_{m}")
                            for hf in range(2):
                                nc.vector.bn_stats(out=st[:, hf, :],
                                                   in_=hp[:, ts(hf, 512)])
                            mv = scal.tile([P, 2], F32, tag="mv", name=f"mv{c}_{m}")
                            nc.vector.bn_aggr(out=mv, in_=st)
                            rstd = scal.tile([P, 1], F32, tag="rstd",
                                             name=f"rstd{c}_{m}")
                            nc.scalar.activation(out=rstd, in_=mv[:, 1:2],
                                                 func=AF.Sqrt, bias=eps_t)
                            nc.vector.reciprocal(rstd, rstd)
                            nc.vector.tensor_scalar(out=hp, in0=hp,
                                                    scalar1=mv[:, 0:1], scalar2=rstd,
                                                    op0=ALU.subtract, op1=ALU.mult)
                            hp_cur.append(hp)
                            # natural-layout affine residual: g1*t1 + (be1+b2)
                            hrm = f32sp.tile([P, D], F32, tag="f32",
                                             name=f"hr{c}_{m}")
                            nc.gpsimd.tensor_mul(hrm, hp, g1bc)
                            nc.gpsimd.tensor_add(hrm, hrm, c1bc)
                            hr_cur.append(hrm)
                    if c > 0:
                        cp = c - 1
                        # w2n prefetch first: FFN1 provides ~14us of lead time
                        w2n = []
                        for n in range(NH):
                            t = w2sp.tile([P, D], BF16, tag="w2", name=f"w2_{cp}_{n}")
                            nc.sync.dma_start(out=t, in_=W2T[ts(n, P), :])
                            w2n.append(t)
                        # ---- FFN1(cp) ----
                        ut = []
                        for n in range(NH):
                            ps = p256.tile([P, CW], F32, tag="mm", name=f"ups{cp}_{n}")
                            for j in range(ND):
                                nc.tensor.matmul(ps, w1p[:, j, ts(n, P)], hc_prev[j],
                                                 start=(j == 0), stop=(j == ND - 1))
                            t = uTp.tile([P, CW], BF16, tag="ut", name=f"ut{cp}_{n}")
                            nc.scalar.activation(out=t, in_=ps, func=AF.Relu,
                                                 bias=b1col[n])
                            ut.append(t)
                        # ---- FFN2(cp): 4 concurrent psums (dc x m) ----
                        psm = [[p512.tile([P, 512], F32, tag="mm",
                                          name=f"fps{cp}_{dc}_{m}")
                                for m in range(M2)] for dc in range(2)]
                        for n in range(NH):
                            for dc in range(2):
                                for m in range(M2):
                                    nc.tensor.matmul(psm[dc][m], ut[n][:, ts(m, P)],
                                                     w2n[n][:, ts(dc, 512)],
                                                     start=(n == 0),
                                                     stop=(n == NH - 1))
                        u2 = [f32sp.tile([P, D], F32, tag="f32", name=f"u2{cp}_{m}")
                              for m in range(M2)]
                        for dc in range(2):
                            for m in range(M2):
                                nc.vector.scalar_tensor_tensor(
                                    out=u2[m][:, ts(dc, 512)], in0=psm[dc][m],
                                    scalar=1.0, in1=hr_prev[m][:, ts(dc, 512)],
                                    op0=ALU.mult, op1=ALU.add)
                        for m in range(M2):
                            sq = cp * M2 + m
                            st = stats.tile([P, 2, 6], F32, tag="st",
                                            name=f"st2{cp}_{m}")
                            for hf in range(2):
                                nc.vector.bn_stats(out=st[:, hf, :],
                                                   in_=u2[m][:, ts(hf, 512)])
                            mv = scal.tile([P, 2], F32, tag="mv", name=f"mv2{cp}_{m}")
                            nc.vector.bn_aggr(out=mv, in_=st)
                            rstd = scal.tile([P, 1], F32, tag="rstd",
                                             name=f"rstd2{cp}_{m}")
                            nc.scalar.activation(out=rstd, in_=mv[:, 1:2],
                                                 func=AF.Sqrt, bias=eps_t)
                            nc.vector.reciprocal(rstd, rstd)
                            nc.vector.tensor_scalar(out=u2[m], in0=u2[m],
                                                    scalar1=mv[:, 0:1], scalar2=rstd,
                                                    op0=ALU.subtract, op1=ALU.mult)
                            ot = f32sp.tile([P, D], F32, tag="f32", name=f"ot{cp}_{m}")
                            # last body: DVE for the tail affine (gpsimd is ~3x
                            # slower and would sit on the critical path)
                            eng = nc.vector if cp == NCH - 1 else nc.gpsimd
                            eng.tensor_mul(ot, u2[m], g2bc)
                            eng.tensor_add(ot, ot, be2bc)
                            nc.sync.dma_start(out=out[ts(sq, P), :], in_=ot)
                    if c < NCH:
                        # ---- hT transposes (LN1 affine folded into DVE copy);
                        # emitted after FFN(c-1) so the LN1 chain is hidden ----
                        hc = [hTcp.tile([P, CW], BF16, tag="hc", name=f"hc{c}_{j}")
                              for j in range(ND)]
                        for m in range(M2):
                            for j in range(ND):
                                if c == NCH - 1:
                                    pst = p512.tile([P, P], F32, tag="mm",
                                                    name=f"tr{c}_{m}_{j}")
                                else:
                                    pst = ptr.tile([P, P], F32, tag="tr",
                                                   name=f"tr{c}_{m}_{j}")
                                nc.tensor.transpose(pst, hp_cur[m][:, ts(j, P)], ident)
                                if c == NCH - 1:
                                    # last chunk: DVE is busy with LN2(c-1);
                                    # use idle ScalarE so the tail FFN isn't
                                    # gated on these copies
                                    nc.scalar.activation(out=hc[j][:, ts(m, P)],
                                                         in_=pst, func=AF.Identity,
                                                         bias=be1col[j],
                                                         scale=g1col[j])
                                else:
                                    nc.vector.tensor_scalar(out=hc[j][:, ts(m, P)],
                                                            in0=pst,
                                                            scalar1=g1col[j],
                                                            scalar2=be1col[j],
                                                            op0=ALU.mult,
                                                            op1=ALU.add)
                        hc_prev, hr_prev = hc, hr_cur

    nc.compile()
    return nc


_CACHE = {}


def _get_nc(S):
    if S not in _CACHE:
        _CACHE[S] = build_nc(S)
    return _CACHE[S]


def kernel(x, Wq, Wk, Wv, W1, b1, W2, b2, g1, be1, g2, be2):
    x = np.asarray(x, np.float32)
    B, S, D_ = x.shape
    nc = _get_nc(S)

    def bft(a):  # transpose + cast to bf16, contiguous
        return np.ascontiguousarray(np.asarray(a, np.float32).T).astype(BF16_NP)

    Gm = (np.asarray(Wk, np.float32).T @ np.asarray(Wq, np.float32)).astype(BF16_NP)
    shared = {
        "G": Gm, "WvT": bft(Wv), "W1T": bft(W1), "W2T": bft(W2),
        "b1": np.asarray(b1, np.float32),
        "g1": np.asarray(g1, np.float32),
        "be1": np.asarray(be1, np.float32),
        "c1": np.asarray(be1, np.float32) + np.asarray(b2, np.float32),
        "g2": np.asarray(g2, np.float32),
        "be2": np.asarray(be2, np.float32),
    }
    in_maps = []
    for b in range(B):
        m = dict(shared)
        m["x_res"] = np.ascontiguousarray(x[b])
        m["xT"] = bft(x[b])
        in_maps.append(m)

    res = run_bass_kernel_spmd(nc, in_maps, core_ids=list(range(B)))
    return np.stack([np.asarray(res.results[b]["out"], np.float32)
                     for b in range(B)], axis=0)
